# revision 1
# baseline (speedup 1.0000x reference)
"""MultiHeadAttention (faithful raw-reshape variant) on 8 trn2 NeuronCores.

Math (per batch b):
  Y  = Xq @ Wq.T            [S, D]
  Z  = Xk @ Wk.T            [S, D]
  V  = Xv @ Wv.T            [S, D]
  reshape (B,S,D)->(B,H,S,dk) is a *raw view*: head h <- rows [128h, 128h+128)
  of Y/Z/V; within the block, q = 16t + j maps to (row t, features 64j..64j+64).
  A  = softmax(Qh @ Kh.T / 8), O = A @ Vh, placed back into the same raw view,
  out = Hcat @ Wo.T + b_o.

Because heads partition the *rows* of Y/Z/V, the computation is fully
independent across (b, h): 32 tasks, 4 per core, no collectives.

Per-core device program (heads hl=0..3 over the core's 512 rows):
  QT/KT: transposed projections  QT[f, s] (f on partitions, 8x128 chunks)
  V:     normal orientation with a 32-wide ones block -> [128, 16, 96]
  scores (transposed): AT[t', q] = sum_k KT[64j'+k, t'] QT[64j+k, t];
         query blocks of opposite j-parity run as pairs on PE row groups
         0-63 / 64-127 (row tiling -> concurrent, 2x score throughput)
  exp on ACT with scale=1/8 fused; no max subtraction (scores ~N(0,1), fp32
         exp cannot overflow); one ACTIVATE per PSUM bank (2-bank reads hang)
  AV:    O.T[k', q] += V[:, j', :].T @ expAT ; rows 64-95 = denominator copies
  normalize: DVE reciprocal + 32-wide quadrant-aligned multiplies into HcatT
  out:   out[t, f'] = HcatT.T @ WoT + b_o

Fallback: TRN_MM_DTYPE=f32 env selects exact fp32 matmuls (~1.5x slower).
"""

import os

import numpy as np

import concourse.bass as bass
import concourse.mybir as mybir
import concourse.tile as tile
from concourse import bacc

B, S, D = 2, 2048, 1024
H, DK = 16, 64
NCORES = 8
HPC = H // (NCORES // B)  # heads per core = 4
SC = HPC * 128            # s-rows per core = 512
P = 128
KD = D // P               # 8 contraction chunks
PO = D // P               # 8 feature chunks
F32 = mybir.dt.float32

# matmul dtype mode: "f32" | "f32r" | "bf16"
MODE = os.environ.get("TRN_MM_DTYPE", "bf16")


def _mm_dt():
    return {"f32": mybir.dt.float32, "f32r": mybir.dt.float32,
            "bf16": mybir.dt.bfloat16}[MODE]


def _np_dt():
    import ml_dtypes
    return {"f32": np.float32, "f32r": np.float32,
            "bf16": ml_dtypes.bfloat16}[MODE]


def _c(ap):
    """Bitcast matmul operands to float32r in f32r mode."""
    if MODE == "f32r":
        return ap.bitcast(mybir.dt.float32r)
    return ap


def build_body(nc, out_ap, ins):
    """Emit the per-core program. ins: dict of DRAM APs."""
    xqt, xkt, xvt = ins["xqt"], ins["xkt"], ins["xvt"]
    wqt, wkt, wvt, wot = ins["wqt"], ins["wkt"], ins["wvt"], ins["wot"]
    bo = ins["bo"]
    mdt = _mm_dt()
    EXPF = mybir.ActivationFunctionType.Exp
    MULT = mybir.AluOpType.mult
    ADD = mybir.AluOpType.add

    wide = MODE == "bf16"  # fp32 storage doesn't fit double-buffered weights
    with tile.TileContext(nc) as tc:
        with (
            tc.tile_pool(name="singles", bufs=1) as singles,
            tc.tile_pool(name="wp", bufs=2 if wide else 1) as wp,
            tc.tile_pool(name="xp", bufs=2) as xp,
            tc.tile_pool(name="exp", bufs=10 if wide else 4) as exp_pool,
            tc.tile_pool(name="smalls", bufs=3) as smalls,
            tc.tile_pool(name="outp", bufs=4) as outp,
            tc.tile_pool(name="ps_mm", bufs=2, space="PSUM") as ps_mm,
            tc.tile_pool(name="ps_at", bufs=4, space="PSUM") as ps_at,
            tc.tile_pool(name="ps_o", bufs=2, space="PSUM") as ps_o,
        ):
            # --- constants ---
            bo_sb = singles.tile([P, D], F32, tag="bo", name="bo_sb")
            bo_bcast = bass.AP(tensor=bo.tensor, offset=bo.offset,
                               ap=[[0, P], list(bo.ap[-1])])
            nc.gpsimd.dma_start(out=bo_sb, in_=bo_bcast)

            qt_sb = singles.tile([P, PO, SC], mdt, tag="qt", name="qt_sb")
            kt_sb = singles.tile([P, PO, SC], mdt, tag="kt", name="kt_sb")
            # partition-rotated copy: kt2[pi] = kt[(pi+64) % 128], so a key
            # slice of either j'-parity is available at either partition base
            kt2_sb = singles.tile([P, PO, SC], mdt, tag="kt2", name="kt2_sb")
            hcat = singles.tile([P, PO, SC], mdt, tag="hcat", name="hcat")
            # [V | ones*32]: the A@V matmul then emits 32 copies of the
            # softmax denominator on partitions 64..95 (time is free: matmul
            # cost depends only on the moving-operand free size)
            v_sb = [singles.tile([P, 16, 96], mdt, tag=f"v{hl}", name=f"v_sb{hl}")
                    for hl in range(HPC)]
            for hl in range(HPC):
                nc.vector.memset(v_sb[hl][:, :, 64:96], 1.0)

            # --- projections ---
            # loads split by kd quarters: region-level deps let the first
            # projection matmuls start after 1/4 of the tensor lands
            def load_w(ap):
                t = wp.tile([P, KD, D], mdt, tag="w", name="w_t")
                src = ap.rearrange("(kd p) f -> p kd f", p=P)
                for q in range(0, KD, 2):
                    nc.sync.dma_start(t[:, q:q + 2], src[:, q:q + 2])
                return t

            def load_x(ap):
                t = xp.tile([P, KD, SC], mdt, tag="x", name="x_t")
                src = ap.rearrange("(kd p) s -> p kd s", p=P)
                for q in range(0, KD, 2):
                    nc.sync.dma_start(t[:, q:q + 2], src[:, q:q + 2])
                return t

            # Q/K transposed: QT[f, s] = sum_d WqT[d, f] XqT[d, s]
            for w_ap, x_ap, dst in ((wqt, xqt, qt_sb), (wkt, xkt, kt_sb)):
                w_t, x_t = load_w(w_ap), load_x(x_ap)
                for mf in range(PO):
                    ps = ps_mm.tile([P, SC], F32, tag="mm", name="ps")
                    for kd in range(KD):
                        nc.tensor.matmul(
                            ps, _c(w_t[:, kd, mf * P:(mf + 1) * P]),
                            _c(x_t[:, kd, :]),
                            start=(kd == 0), stop=(kd == KD - 1))
                    nc.vector.tensor_copy(dst[:, mf, :], ps)
                    if dst is kt_sb:
                        nc.sync.dma_start(kt2_sb[0:64, mf],
                                          kt_sb[64:128, mf])
                        nc.sync.dma_start(kt2_sb[64:128, mf],
                                          kt_sb[0:64, mf])

            # V normal: V[s, f] = sum_d XvT[d, s] WvT[d, f].
            # Only head 0's V up front; heads 1-3 are emitted after head 0's
            # first attention half so PE feeds ACT scores sooner (emission
            # order is dependency order in Tile, so v_proj(h) must still
            # precede head h's first A@V matmul).
            wv_t, xv_t = load_w(wvt), load_x(xvt)

            def v_proj(hl):
                for nf in range(2):
                    ps = ps_mm.tile([P, SC], F32, tag="mm", name="ps")
                    for kd in range(KD):
                        nc.tensor.matmul(
                            ps, _c(xv_t[:, kd, hl * P:(hl + 1) * P]),
                            _c(wv_t[:, kd, nf * 512:(nf + 1) * 512]),
                            start=(kd == 0), stop=(kd == KD - 1))
                    nc.vector.tensor_copy(
                        v_sb[hl][:, nf * 8:(nf + 1) * 8, 0:64],
                        ps.rearrange("p (j k) -> p j k", k=64))

            v_proj(0)

            wo_t = load_w(wot)  # prefetched during attention

            # --- attention per local head ---
            # Query blocks of opposite j-parity are processed in pairs: their
            # score matmuls run on PE row-groups 0-63 / 64-127 and execute
            # concurrently (row tiling), doubling score throughput.
            # outproj(h) is emitted after head h+1's first half so the next
            # head's scores reach PE at the boundary and ACT never starves.
            pending = []
            for hl in range(HPC):
                hs = slice(hl * P, (hl + 1) * P)
                for pp in range(2):  # po-half; qbA has a=0, qbB a=1
                    rhs_a = qt_sb[0:64, 4 * pp:4 * pp + 4, hs]
                    rhs_b = qt_sb[64:128, 4 * pp:4 * pp + 4, hs]
                    o_a = ps_o.tile([96, 512], F32, tag="o", name="o_a")
                    o_b = ps_o.tile([96, 512], F32, tag="o", name="o_b")
                    for jp in range(16):
                        a2, po2 = jp % 2, jp // 2
                        ksrc_a = kt_sb if a2 == 0 else kt2_sb
                        ksrc_b = kt_sb if a2 == 1 else kt2_sb
                        at_a = ps_at.tile([P, 512], F32, tag="at", name="at_a")
                        at_b = ps_at.tile([P, 512], F32, tag="at", name="at_b")
                        # adjacent matmuls on PE row groups 0-63 / 64-127
                        # execute concurrently (row tiling)
                        nc.tensor.matmul(at_a, _c(ksrc_a[0:64, po2, hs]),
                                         _c(rhs_a), start=True, stop=True)
                        nc.tensor.matmul(at_b, _c(ksrc_b[64:128, po2, hs]),
                                         _c(rhs_b), start=True, stop=True)
                        ex_a = exp_pool.tile([P, 512], mdt, tag="ex",
                                             name="ex_a")
                        ex_b = exp_pool.tile([P, 512], mdt, tag="ex",
                                             name="ex_b")
                        # note: a single ACTIVATE must not read >1 PSUM bank
                        # (2-bank reads hang the device)
                        nc.scalar.activation(ex_a, at_a, EXPF, scale=0.125)
                        nc.scalar.activation(ex_b, at_b, EXPF, scale=0.125)
                        st, sp = jp == 0, jp == 15
                        nc.tensor.matmul(o_a, _c(v_sb[hl][:, jp, :]),
                                         _c(ex_a), start=st, stop=sp)
                        nc.tensor.matmul(o_b, _c(v_sb[hl][:, jp, :]),
                                         _c(ex_b), start=st, stop=sp)
                    # normalize into HcatT: recip of the replicated denom
                    # rows, then 32-wide multiplies (quadrant-aligned)
                    for a, o_ps in ((0, o_a), (1, o_b)):
                        rc = smalls.tile([P, 512], F32, tag="rc", name="rc")
                        nc.vector.reciprocal(rc[64:96, :], o_ps[64:96, :])
                        dst = hcat[64 * a:64 * a + 64, 4 * pp:4 * pp + 4, hs]
                        for u in range(2):
                            nc.vector.tensor_tensor(
                                dst[32 * u:32 * u + 32],
                                o_ps[32 * u:32 * u + 32, :].rearrange(
                                    "k (c t) -> k c t", t=P),
                                rc[64:96, :].rearrange("k (c t) -> k c t", t=P),
                                MULT)

                    if pp == 0:
                        if hl == 0:
                            for h2 in range(1, HPC):
                                v_proj(h2)
                        for emit in pending:
                            emit()
                        pending = []

                # output projection for this head block (deferred emission)
                def outproj(hs=hs):
                    for nf in range(2):
                        fs = slice(nf * 512, (nf + 1) * 512)
                        ps = ps_mm.tile([P, 512], F32, tag="mm", name="ps")
                        for po in range(PO):
                            nc.tensor.matmul(
                                ps, _c(hcat[:, po, hs]), _c(wo_t[:, po, fs]),
                                start=(po == 0), stop=(po == PO - 1))
                        os_t = outp.tile([P, 512], F32, tag="os", name="os_t")
                        nc.vector.tensor_tensor(os_t, ps, bo_sb[:, fs], ADD)
                        nc.sync.dma_start(out_ap[hs, fs], os_t)
                pending.append(outproj)

            for emit in pending:
                emit()
    return nc


def build_program():
    nc = bacc.Bacc("TRN2", target_bir_lowering=False, debug=False,
                   enable_asserts=False, num_devices=NCORES)
    mdt = _mm_dt()
    ins = {
        "xqt": nc.dram_tensor("xqt", [D, SC], mdt, kind="ExternalInput").ap(),
        "xkt": nc.dram_tensor("xkt", [D, SC], mdt, kind="ExternalInput").ap(),
        "xvt": nc.dram_tensor("xvt", [D, SC], mdt, kind="ExternalInput").ap(),
        "wqt": nc.dram_tensor("wqt", [D, D], mdt, kind="ExternalInput").ap(),
        "wkt": nc.dram_tensor("wkt", [D, D], mdt, kind="ExternalInput").ap(),
        "wvt": nc.dram_tensor("wvt", [D, D], mdt, kind="ExternalInput").ap(),
        "wot": nc.dram_tensor("wot", [D, D], mdt, kind="ExternalInput").ap(),
        "bo": nc.dram_tensor("bo", [1, D], F32, kind="ExternalInput").ap(),
    }
    out_ap = nc.dram_tensor("out", [SC, D], F32, kind="ExternalOutput").ap()
    build_body(nc, out_ap, ins)
    nc.finalize()
    return nc


def make_in_maps(inputs):
    ndt = _np_dt()
    Xq = np.asarray(inputs["X_q"], dtype=np.float32)
    Xk = np.asarray(inputs["X_k"], dtype=np.float32)
    Xv = np.asarray(inputs["X_v"], dtype=np.float32)
    wqt = np.ascontiguousarray(np.asarray(inputs["W_q"], np.float32).T).astype(ndt)
    wkt = np.ascontiguousarray(np.asarray(inputs["W_k"], np.float32).T).astype(ndt)
    wvt = np.ascontiguousarray(np.asarray(inputs["W_v"], np.float32).T).astype(ndt)
    wot = np.ascontiguousarray(np.asarray(inputs["W_o"], np.float32).T).astype(ndt)
    bo = np.asarray(inputs["b_o"], np.float32).reshape(1, D)
    xt = {n: [np.ascontiguousarray(x[b].T).astype(ndt) for b in range(B)]
          for n, x in (("xqt", Xq), ("xkt", Xk), ("xvt", Xv))}
    in_maps = []
    for c in range(NCORES):
        b, g = divmod(c, NCORES // B)
        sl = slice(g * SC, (g + 1) * SC)
        in_maps.append({
            "xqt": np.ascontiguousarray(xt["xqt"][b][:, sl]),
            "xkt": np.ascontiguousarray(xt["xkt"][b][:, sl]),
            "xvt": np.ascontiguousarray(xt["xvt"][b][:, sl]),
            "wqt": wqt, "wkt": wkt, "wvt": wvt, "wot": wot, "bo": bo,
        })
    return in_maps


_NC_CACHE = {}


def _run(inputs, trace=False, trace_cores=None):
    from concourse.bass_utils import run_bass_kernel_spmd
    if MODE not in _NC_CACHE:
        _NC_CACHE[MODE] = build_program()
    nc = _NC_CACHE[MODE]
    in_maps = make_in_maps(inputs)
    res = run_bass_kernel_spmd(nc, in_maps, core_ids=list(range(NCORES)),
                               trace=trace, trace_cores=trace_cores)
    out = np.empty((B, S, D), dtype=np.float32)
    for c in range(NCORES):
        b, g = divmod(c, NCORES // B)
        out[b, g * SC:(g + 1) * SC, :] = res.results[c]["out"]
    return out, res


def kernel(**inputs):
    out, _ = _run(inputs, trace=False)
    return out



# revision 33
# speedup vs baseline: 1.2038x; 1.2038x over previous
"""MultiHeadAttention (faithful raw-reshape variant) on 8 trn2 NeuronCores.

Math (per batch b):
  Y  = Xq @ Wq.T            [S, D]
  Z  = Xk @ Wk.T            [S, D]
  V  = Xv @ Wv.T            [S, D]
  reshape (B,S,D)->(B,H,S,dk) is a *raw view*: head h <- rows [128h, 128h+128)
  of Y/Z/V; within the block, q = 16t + j maps to (row t, features 64j..64j+64).
  A  = softmax(Qh @ Kh.T / 8), O = A @ Vh, placed back into the same raw view,
  out = Hcat @ Wo.T + b_o.

Heads partition the *rows* of Y/Z/V, so work is fully independent across
(b, h): 32 tasks, 4 per core, no collectives.

Per-core device program (heads hl=0..3 over the core's 512 rows):
  QT/KT: transposed projections QT[f, s] ([128, 8, 512]); kt2 = partition-
         rotated KT copy so either j'-parity is on either partition half.
  scores: at[t', 128c+t] = sum_d K[(t',j'), d] Q[(t,j(c)), d]; one matmul per
         (head, quarter, j'): stationary kt-chunk [64, 128], moving qt [64,512].
  exp:   ACT, scale=1/8 fused, fp32 PSUM -> bf16 SBUF. ACT is the bottleneck
         engine (256 x ~0.6us); the schedule keeps it saturated.
  AV:    O[q, f] orientation: stationary ex-chunk [128, 128], moving
         V-chunk [128, 65] (64 V cols + ones col -> softmax denominator on
         free idx 64). 16 j'-chunks accumulate per PSUM tile [128, 4, 128].
         AVs lag scores by one quarter so PE never waits on ACT results.
  norm:  DVE reciprocal of denom cols + per-partition tensor_scalar multiply
         into bf16 hcp tiles [128 t, 2 side, 64 d] (= Hcat f-chunks).
  transpose: PE transpose (via identity) hcp -> hcatT chunks [128 f, 128 t].
  out:   out[t, g] = hcatT.T @ WoT + b_o; stores ride the DVE queue.
"""

import os

import numpy as np

import concourse.bass as bass
import concourse.mybir as mybir
import concourse.tile as tile
from concourse import bacc
from concourse.masks import make_identity

B, S, D = 2, 2048, 1024
H, DK = 16, 64
NCORES = 8
HPC = H // (NCORES // B)  # heads per core = 4
SC = HPC * 128            # s-rows per core = 512
P = 128
KD = D // P               # 8 contraction chunks
PO = D // P               # 8 feature chunks
F32 = mybir.dt.float32
BF16 = mybir.dt.bfloat16

MODE = "bf16"
N_WARMUP = int(os.environ.get("TRN_WARMUP", "14"))


def build_body(nc, out_ap, ins):
    """Emit the per-core program. ins: dict of DRAM APs."""
    EXPF = mybir.ActivationFunctionType.Exp
    MULT = mybir.AluOpType.mult
    ADD = mybir.AluOpType.add

    with tile.TileContext(nc) as tc:
        with (
            tc.tile_pool(name="singles", bufs=1) as singles,
            tc.tile_pool(name="exq", bufs=36) as exq,
            tc.tile_pool(name="hcp", bufs=8) as hcpp,
            tc.tile_pool(name="outp", bufs=4) as outp,
            tc.tile_pool(name="ps_at", bufs=3, space="PSUM") as ps_at,
            tc.tile_pool(name="ps_o", bufs=2, space="PSUM") as ps_o,
            tc.tile_pool(name="ps_tr", bufs=1, space="PSUM") as ps_tr,
            tc.tile_pool(name="ps_mm", bufs=2, space="PSUM") as ps_mm,
        ):
            # ---------------- constants / singles ----------------
            bo_sb = singles.tile([P, D], F32, tag="bo", name="bo_sb")
            bo = ins["bo"]
            bo_bcast = bass.AP(tensor=bo.tensor, offset=bo.offset,
                               ap=[[0, P], list(bo.ap[-1])])
            nc.gpsimd.dma_start(out=bo_sb, in_=bo_bcast)

            ident = singles.tile([P, P], BF16, tag="id", name="ident")
            make_identity(nc, ident)

            qt = singles.tile([P, PO, SC], BF16, tag="qt", name="qt")
            kt = singles.tile([P, PO, SC], BF16, tag="kt", name="kt")
            kt2 = singles.tile([P, PO, SC], BF16, tag="kt2", name="kt2")
            hcatT = singles.tile([P, PO, SC], BF16, tag="hct", name="hcatT")
            # V per head: [t', j', 64 V cols + ones col] -> denominator rides
            # the A@V matmul on free idx 64
            v_sb = [singles.tile([P, 16, 65], BF16, tag=f"v{hl}",
                                 name=f"v_sb{hl}") for hl in range(HPC)]
            for hl in range(HPC):
                nc.gpsimd.memset(v_sb[hl][:, :, 64:65], 1.0)

            # warmup fodder (keeps the PE p-state ramp warm during DMA waits)
            junk = singles.tile([P, 512], BF16, tag="junk", name="junk")
            nc.gpsimd.memset(junk, 0.0)

            # ---------------- input DMA emissions ----------------
            # Emission order == DMA service order (one pooled DMA device in
            # the cost model); order the critical first-scores chain first.
            xs = {}   # (name, s-half) -> tile  [P, KD, 256]
            ws = {}   # name -> tile [P, KD, D], loaded in f-halves
            xsrc = {"xq": ins["xqt"], "xk": ins["xkt"], "xv": ins["xvt"]}
            wsrc = {"wq": ins["wqt"], "wk": ins["wkt"], "wv": ins["wvt"],
                    "wo": ins["wot"]}
            for n, ap in wsrc.items():
                ws[n] = singles.tile([P, KD, D], BF16, tag=n, name=n)
            for n in xsrc:
                for sh in range(2):
                    xs[n, sh] = singles.tile([P, KD, 256], BF16,
                                             tag=f"{n}{sh}", name=f"{n}{sh}")

            def load_w(n, fh, quarter=False):
                src = wsrc[n].rearrange("(kd p) f -> p kd f", p=P)
                w = 256 if quarter else 512
                fs = slice(fh * w, (fh + 1) * w)
                nc.sync.dma_start(ws[n][:, :, fs], src[:, :, fs])

            def load_x(n, sh):
                src = xsrc[n].rearrange("(kd p) s -> p kd s", p=P)
                ss = slice(sh * 256, (sh + 1) * 256)
                nc.sync.dma_start(xs[n, sh], src[:, :, ss])

            def load_kt2(mlo, mhi, sh):
                """Grouped partition-rotated KT copy (kt2[p]=kt[(p+64)%128]).
                Placed in the SP DMA stream: its kt-copy waits block SP's
                in-order SEQ, which both throttles later (non-urgent) input
                loads and guarantees kt2 isn't starved on the DMA device."""
                ms = slice(mlo, mhi)
                ss = slice(sh * 256, (sh + 1) * 256)
                nc.sync.dma_start(kt2[0:64, ms, ss], kt[64:128, ms, ss])
                nc.sync.dma_start(kt2[64:128, ms, ss], kt[0:64, ms, ss])

            # critical chain first: first scores need wq-h0 + xq-h0 (Q mf0-3)
            # + wk quarters streaming. All remaining loads are emitted inside
            # quarter 0 (after the kt2-sh0 rotation DMAs) so the kt2 copies
            # get SP-queue priority over the long-slack input loads. kt2
            # emissions must FOLLOW their kt-copy producers (Tile deps track
            # emission order).
            load_w("wq", 0, True); load_x("xq", 0)
            load_w("wq", 1, True)
            load_x("xk", 0)
            load_w("wk", 0, True); load_w("wk", 1, True)
            load_w("wk", 2, True); load_w("wk", 3, True)

            def load_rest():
                load_w("wq", 1)
                load_w("wv", 0); load_x("xv", 0); load_w("wv", 1)
                load_x("xq", 1); load_x("xk", 1); load_x("xv", 1)
                load_w("wo", 0); load_w("wo", 1)

            # ---------------- PE warmup ----------------
            for _ in range(N_WARMUP):
                d_ps = ps_at.tile([P, 512], F32, tag="at", name="d_ps")
                nc.tensor.matmul(d_ps, junk[:, 0:P], junk, start=True,
                                 stop=True)

            # ---------------- projection units ----------------
            # proj_qk(t, mf, sh): QT/KT[f-chunk mf, s-half sh] = 8 kd matmuls
            # (free 256) + DVE copy; K also emits the kt2 rotated copy (SWDGE).
            def proj_qk(which, mf, sh):
                w_t = ws["wq" if which == "q" else "wk"]
                x_t = xs[("xq" if which == "q" else "xk"), sh]
                dst = qt if which == "q" else kt
                ss = slice(sh * 256, (sh + 1) * 256)
                ps = ps_mm.tile([P, 512], F32, tag="mm", name="ps")
                for kd in range(KD):
                    nc.tensor.matmul(
                        ps[:, 0:256], w_t[:, kd, mf * P:(mf + 1) * P],
                        x_t[:, kd, :], start=(kd == 0), stop=(kd == KD - 1))
                nc.vector.tensor_copy(dst[:, mf, ss], ps[:, 0:256])

            # V proj: V[s, f] for head hl, f-half nf (j' chunks 8nf..8nf+7)
            def v_proj(hl, nf):
                sh, so = hl // 2, (hl % 2) * 128
                ps = ps_mm.tile([P, 512], F32, tag="mm", name="ps")
                for kd in range(KD):
                    nc.tensor.matmul(
                        ps, xs["xv", sh][:, kd, so:so + P],
                        ws["wv"][:, kd, nf * 512:(nf + 1) * 512],
                        start=(kd == 0), stop=(kd == KD - 1))
                nc.vector.tensor_copy(
                    v_sb[hl][:, nf * 8:(nf + 1) * 8, 0:64],
                    ps.rearrange("p (j k) -> p j k", k=64))

            # ---------------- attention pieces ----------------
            # per (head, quarter qi): quarter = (pp, a); j'-order: same-parity
            # first (kt source) then opposite (kt2, which lands ~2us later).
            def jorder(a):
                return [j for j in range(16) if j % 2 == a] + \
                       [j for j in range(16) if j % 2 != a]

            ex_tiles = {}  # (hl, qi, j') -> SBUF bf16 [128, 512]

            def scores_exp(hl, qi, jp):
                pp, a = qi // 2, qi % 2
                hs = slice(hl * P, (hl + 1) * P)
                pb = slice(64 * a, 64 * a + 64)
                src = kt if jp % 2 == a else kt2
                at = ps_at.tile([P, 512], F32, tag="at", name="at")
                nc.tensor.matmul(at, src[pb, jp // 2, hs],
                                 qt[pb, 4 * pp:4 * pp + 4, hs],
                                 start=True, stop=True)
                ex = exq.tile([P, 512], BF16, tag="ex", name="ex")
                nc.scalar.activation(ex, at, EXPF, scale=0.125)
                ex_tiles[hl, qi, jp] = ex

            o_tiles = {}  # (hl, qi) -> PSUM [128, 4, 128]

            def av(hl, qi, jp, first, last):
                # one accumulation group per o4 bank: start zeroes the whole
                # 2KB zero-region lazily, so only the very first matmul may
                # set start and only the very last sets stop
                ex = ex_tiles.pop((hl, qi, jp))
                if first:
                    o_tiles[hl, qi] = ps_o.tile([P, 4, P], F32, tag="o",
                                                name="o4")
                o4 = o_tiles[hl, qi]
                for c in range(4):
                    nc.tensor.matmul(o4[:, c, 0:65],
                                     ex[:, c * P:(c + 1) * P],
                                     v_sb[hl][:, jp, :],
                                     start=first and c == 0,
                                     stop=last and c == 3)

            hcp_tiles = {}  # (hl, po) -> SBUF bf16 [128, 2, 64]

            def norm(hl, qi):
                """recip + 4 per-partition scaling multiplies for quarter qi."""
                pp, a = qi // 2, qi % 2
                o4 = o_tiles.pop((hl, qi))
                nc.vector.reciprocal(o4[:, :, 64:65], o4[:, :, 64:65])
                for c in range(4):
                    po = 4 * pp + c
                    if (hl, po) not in hcp_tiles:
                        hcp_tiles[hl, po] = hcpp.tile([P, 2, 64], BF16,
                                                      tag="hcp", name="hcp")
                    nc.vector.tensor_scalar(
                        hcp_tiles[hl, po][:, a, :], o4[:, c, 0:64],
                        o4[:, c, 64:65], None, MULT)

            def transposes(hl, pp):
                hs = slice(hl * P, (hl + 1) * P)
                tps = ps_tr.tile([P, 4, P], BF16, tag="tr", name="tps")
                for c in range(4):
                    po = 4 * pp + c
                    hcp = hcp_tiles.pop((hl, po))
                    nc.tensor.transpose(tps[:, c, :],
                                        hcp.rearrange("p a d -> p (a d)"),
                                        ident)
                    nc.vector.tensor_copy(hcatT[:, po, hs], tps[:, c, :])

            op_ps = {}

            def outproj(hl, nf, poh):
                """Half of the po-contraction for (head, f-half nf)."""
                hs = slice(hl * P, (hl + 1) * P)
                fs = slice(nf * 512, (nf + 1) * 512)
                if poh == 0:
                    op_ps[hl, nf] = ps_mm.tile([P, 512], F32, tag="mm",
                                               name="ps")
                ps = op_ps[hl, nf]
                for po in range(4 * poh, 4 * poh + 4):
                    nc.tensor.matmul(ps, hcatT[:, po, hs], ws["wo"][:, po, fs],
                                     start=(po == 0), stop=(po == PO - 1))
                if poh == 1:
                    del op_ps[hl, nf]
                    os_t = outp.tile([P, 512], F32, tag="os", name="os_t")
                    nc.vector.tensor_tensor(os_t, ps, bo_sb[:, fs], ADD)
                    nc.sync.dma_start(out_ap[hs, fs], os_t)

            # ---------------- schedule ----------------
            # Pre-attention: Q mf0-3 + K mf0-3 (s-half 0); quarter 0 runs a
            # custom j'-order with K mf4-7 interleaved as their DMA halves
            # land. All other proj / v_proj / outproj work becomes filler
            # units drained inside the attention loop at fixed slots — PE
            # runs ~2x faster than ACT during attention, so fillers absorb
            # PE idle gaps without starving ACT (3 at-banks = ~1.8us of
            # buffered ACT work rides out short PE stalls).
            # AVs lag scores by TWO quarters so every ex tile and V half is
            # long since ready when its AV executes.
            for mf in range(3):
                proj_qk("q", mf, 0)
            proj_qk("k", 0, 0)
            proj_qk("q", 3, 0)

# Deferred work units, drained at fixed slots inside the
            # attention loop (PE outruns ACT there, so units absorb PE idle
            # gaps). Every unit is keyed and ensure()d right before its
            # first consumer is emitted — slot arithmetic is a performance
            # heuristic, never a correctness requirement.
            from collections import OrderedDict
            units = OrderedDict()

            def kp_sh1(mf):
                proj_qk("k", mf, 1)
                if mf == 3:
                    load_kt2(0, 4, 1)
                elif mf == 7:
                    load_kt2(4, 8, 1)

            for mf in range(4, PO):
                units["qp", mf, 0] = lambda mf=mf: proj_qk("q", mf, 0)
            units["vp", 0, 0] = lambda: v_proj(0, 0)
            units["vp", 0, 1] = lambda: v_proj(0, 1)
            units["vp", 1, 0] = lambda: v_proj(1, 0)
            units["vp", 1, 1] = lambda: v_proj(1, 1)
            for mf in range(PO):
                units["kp", mf, 1] = lambda mf=mf: kp_sh1(mf)
                units["qp", mf, 1] = lambda mf=mf: proj_qk("q", mf, 1)
                if mf == 2:
                    units["vp", 2, 0] = lambda: v_proj(2, 0)
                    units["vp", 2, 1] = lambda: v_proj(2, 1)
                if mf == 4:
                    units["vp", 3, 0] = lambda: v_proj(3, 0)
                    units["vp", 3, 1] = lambda: v_proj(3, 1)

            def ensure(*key):
                fn = units.pop(key, None)
                if fn is not None:
                    fn()

            def fill(n=1):
                for _ in range(n):
                    if units:
                        units.pop(next(iter(units)))()

            def ensure_scores_deps(shl, sqi):
                """Emit any still-pending proj units whose outputs quarter
                (shl, sqi)'s scores read."""
                sh = shl // 2
                if sh == 1:
                    for mf in range(PO):
                        ensure("kp", mf, 1)
                mlo = 4 * (sqi // 2)
                for mf in range(mlo, mlo + 4):
                    ensure("qp", mf, sh)

            quarters = [(hl, qi) for hl in range(HPC) for qi in range(4)]
            jo_used = {}

            def post_av_block(ahl, aqi):
                norm(ahl, aqi)
                if aqi % 2 == 1:
                    transposes(ahl, aqi // 2)
                    if aqi == 3:
                        for nf, poh in ((0, 0), (0, 1), (1, 0), (1, 1)):
                            units["op", ahl, nf, poh] = (
                                lambda hl=ahl, nf=nf, poh=poh:
                                outproj(hl, nf, poh))

            for i, (hl, qi) in enumerate(quarters):
                avt = quarters[i - 2] if i >= 2 else None
                jo_av = jo_used[avt] if avt else None
                if i == 0:
                    # custom: quarter 0's kt-parity (even) j' interleaved
                    # with (a) K mf1-7 projections as the wk quarters land
                    # and (b) quarter 1's kt-parity (odd) j' — the K-proj
                    # phase is PE-bound at ~1.06us/score, so doubling the
                    # score stream here keeps ACT saturated. kt2 rotation
                    # DMAs then have a half-quarter of slack before the
                    # first kt2-parity j' needs them.
                    evens = [0, 2, 4, 6, 8, 10, 12, 14]
                    odds = [1, 3, 5, 7, 9, 11, 13, 15]
                    for idx in range(8):
                        scores_exp(hl, 0, evens[idx])
                        if idx < 7:
                            proj_qk("k", idx + 1, 0)
                        if idx == 2:
                            load_kt2(0, 4, 0)
                        elif idx == 6:
                            load_kt2(4, 8, 0)
                        elif idx == 7:
                            load_rest()
                        scores_exp(hl, 1, odds[idx])
                    for jp in odds:
                        scores_exp(hl, 0, jp)
                    jo_used[hl, 0] = evens + odds
                elif i == 1:
                    for idx, jp in enumerate(evens):
                        scores_exp(hl, 1, jp)
                        if idx % 2 == 1:
                            fill()
                    jo_used[hl, 1] = odds + evens
                else:
                    ensure_scores_deps(hl, qi)
                    if avt is not None and avt[1] == 0:
                        ensure("vp", avt[0], 0)
                        ensure("vp", avt[0], 1)
                    jo = jorder(qi % 2)
                    jo_used[hl, qi] = jo
                    for idx, jp in enumerate(jo):
                        scores_exp(hl, qi, jp)
                        if avt is not None:
                            av(*avt, jo_av[idx], idx == 0, idx == 15)
                        if idx % 3 == 2:
                            fill()
                if avt is not None:
                    post_av_block(*avt)

            # ---- tail ----
            # quarter (3,2): AVs + norm; head-3's po0-3 outproj partials can
            # already run (transposes(3,0) done after quarter (3,1)).
            thl, tqi = quarters[-2]
            jo_av = jo_used[thl, tqi]
            for idx, jp in enumerate(jo_av):
                av(thl, tqi, jp, idx == 0, idx == 15)
                fill()
            norm(thl, tqi)
            fill(len(units))
            outproj(thl, 0, 0)
            outproj(thl, 1, 0)

            # quarter (3,3): AVs, then a per-chunk pipelined finale so the
            # post-last-exp critical chain is as short as possible:
            # recip_c -> norm_c -> transpose_c -> copy_c -> outproj(po4+c),
            # then bias + store per f-half.
            thl, tqi = quarters[-1]
            jo_av = jo_used[thl, tqi]
            for idx, jp in enumerate(jo_av):
                av(thl, tqi, jp, idx == 0, idx == 15)
            o4 = o_tiles.pop((thl, tqi))
            hs = slice(thl * P, (thl + 1) * P)
            tps = ps_tr.tile([P, 4, P], BF16, tag="tr", name="tps")
            for c in range(4):
                po = 4 + c
                nc.vector.reciprocal(o4[:, c, 64:65], o4[:, c, 64:65])
                hcp = hcp_tiles.pop((thl, po))
                nc.vector.tensor_scalar(hcp[:, 1, :], o4[:, c, 0:64],
                                        o4[:, c, 64:65], None, MULT)
                nc.tensor.transpose(tps[:, c, :],
                                    hcp.rearrange("p a d -> p (a d)"), ident)
                nc.vector.tensor_copy(hcatT[:, po, hs], tps[:, c, :])
                for nf in range(2):
                    fs = slice(nf * 512, (nf + 1) * 512)
                    nc.tensor.matmul(op_ps[thl, nf], hcatT[:, po, hs],
                                     ws["wo"][:, po, fs],
                                     start=False, stop=(po == PO - 1))
            for nf in range(2):
                fs = slice(nf * 512, (nf + 1) * 512)
                ps = op_ps.pop((thl, nf))
                os_t = outp.tile([P, 512], F32, tag="os", name="os_t")
                nc.vector.tensor_tensor(os_t, ps, bo_sb[:, fs], ADD)
                nc.sync.dma_start(out_ap[hs, fs], os_t)
    return nc


def build_program():
    nc = bacc.Bacc("TRN2", target_bir_lowering=False, debug=False,
                   enable_asserts=False, num_devices=NCORES)
    ins = {
        "xqt": nc.dram_tensor("xqt", [D, SC], BF16, kind="ExternalInput").ap(),
        "xkt": nc.dram_tensor("xkt", [D, SC], BF16, kind="ExternalInput").ap(),
        "xvt": nc.dram_tensor("xvt", [D, SC], BF16, kind="ExternalInput").ap(),
        "wqt": nc.dram_tensor("wqt", [D, D], BF16, kind="ExternalInput").ap(),
        "wkt": nc.dram_tensor("wkt", [D, D], BF16, kind="ExternalInput").ap(),
        "wvt": nc.dram_tensor("wvt", [D, D], BF16, kind="ExternalInput").ap(),
        "wot": nc.dram_tensor("wot", [D, D], BF16, kind="ExternalInput").ap(),
        "bo": nc.dram_tensor("bo", [1, D], F32, kind="ExternalInput").ap(),
    }
    out_ap = nc.dram_tensor("out", [SC, D], F32, kind="ExternalOutput").ap()
    build_body(nc, out_ap, ins)
    nc.finalize()
    return nc


def make_in_maps(inputs):
    import ml_dtypes
    ndt = ml_dtypes.bfloat16
    Xq = np.asarray(inputs["X_q"], dtype=np.float32)
    Xk = np.asarray(inputs["X_k"], dtype=np.float32)
    Xv = np.asarray(inputs["X_v"], dtype=np.float32)
    wqt = np.ascontiguousarray(np.asarray(inputs["W_q"], np.float32).T).astype(ndt)
    wkt = np.ascontiguousarray(np.asarray(inputs["W_k"], np.float32).T).astype(ndt)
    wvt = np.ascontiguousarray(np.asarray(inputs["W_v"], np.float32).T).astype(ndt)
    wot = np.ascontiguousarray(np.asarray(inputs["W_o"], np.float32).T).astype(ndt)
    bo = np.asarray(inputs["b_o"], np.float32).reshape(1, D)
    xt = {n: [np.ascontiguousarray(x[b].T).astype(ndt) for b in range(B)]
          for n, x in (("xqt", Xq), ("xkt", Xk), ("xvt", Xv))}
    in_maps = []
    for c in range(NCORES):
        b, g = divmod(c, NCORES // B)
        sl = slice(g * SC, (g + 1) * SC)
        in_maps.append({
            "xqt": np.ascontiguousarray(xt["xqt"][b][:, sl]),
            "xkt": np.ascontiguousarray(xt["xkt"][b][:, sl]),
            "xvt": np.ascontiguousarray(xt["xvt"][b][:, sl]),
            "wqt": wqt, "wkt": wkt, "wvt": wvt, "wot": wot, "bo": bo,
        })
    return in_maps


_NC_CACHE = {}


def _run(inputs, trace=False, trace_cores=None):
    from concourse.bass_utils import run_bass_kernel_spmd
    if MODE not in _NC_CACHE:
        _NC_CACHE[MODE] = build_program()
    nc = _NC_CACHE[MODE]
    in_maps = make_in_maps(inputs)
    res = run_bass_kernel_spmd(nc, in_maps, core_ids=list(range(NCORES)),
                               trace=trace, trace_cores=trace_cores)
    out = np.empty((B, S, D), dtype=np.float32)
    for c in range(NCORES):
        b, g = divmod(c, NCORES // B)
        out[b, g * SC:(g + 1) * SC, :] = res.results[c]["out"]
    return out, res


def kernel(**inputs):
    out, _ = _run(inputs, trace=False)
    return out


# revision 42
# speedup vs baseline: 1.2092x; 1.0045x over previous
"""MultiHeadAttention (faithful raw-reshape variant) on 8 trn2 NeuronCores.

Math (per batch b):
  Y  = Xq @ Wq.T            [S, D]
  Z  = Xk @ Wk.T            [S, D]
  V  = Xv @ Wv.T            [S, D]
  reshape (B,S,D)->(B,H,S,dk) is a *raw view*: head h <- rows [128h, 128h+128)
  of Y/Z/V; within the block, q = 16t + j maps to (row t, features 64j..64j+64).
  A  = softmax(Qh @ Kh.T / 8), O = A @ Vh, placed back into the same raw view,
  out = Hcat @ Wo.T + b_o.

Heads partition the *rows* of Y/Z/V, so work is fully independent across
(b, h): 32 tasks, 4 per core, no collectives.

Per-core device program (heads hl=0..3 over the core's 512 rows):
  QT/KT: transposed projections QT[f, s] ([128, 8, 512]); kt2 = partition-
         rotated KT copy so either j'-parity is on either partition half.
  scores: at[t', 128c+t] = sum_d K[(t',j'), d] Q[(t,j(c)), d]; one matmul per
         (head, quarter, j'): stationary kt-chunk [64, 128], moving qt [64,512].
  exp:   ACT, scale=1/8 fused, fp32 PSUM -> bf16 SBUF. ACT is the bottleneck
         engine (256 x ~0.6us); the schedule keeps it saturated.
  AV:    O[q, f] orientation: stationary ex-chunk [128, 128], moving
         V-chunk [128, 65] (64 V cols + ones col -> softmax denominator on
         free idx 64). 16 j'-chunks accumulate per PSUM tile [128, 4, 128].
         AVs lag scores by one quarter so PE never waits on ACT results.
  norm:  DVE reciprocal of denom cols + per-partition tensor_scalar multiply
         into bf16 hcp tiles [128 t, 2 side, 64 d] (= Hcat f-chunks).
  transpose: PE transpose (via identity) hcp -> hcatT chunks [128 f, 128 t].
  out:   out[t, g] = hcatT.T @ WoT + b_o; stores ride the DVE queue.
"""

import os

import numpy as np

import concourse.bass as bass
import concourse.mybir as mybir
import concourse.tile as tile
from concourse import bacc
from concourse.masks import make_identity

B, S, D = 2, 2048, 1024
H, DK = 16, 64
NCORES = 8
HPC = H // (NCORES // B)  # heads per core = 4
SC = HPC * 128            # s-rows per core = 512
P = 128
KD = D // P               # 8 contraction chunks
PO = D // P               # 8 feature chunks
F32 = mybir.dt.float32
BF16 = mybir.dt.bfloat16

MODE = "bf16"
N_WARMUP = int(os.environ.get("TRN_WARMUP", "14"))


def build_body(nc, out_ap, ins):
    """Emit the per-core program. ins: dict of DRAM APs."""
    EXPF = mybir.ActivationFunctionType.Exp
    MULT = mybir.AluOpType.mult
    ADD = mybir.AluOpType.add

    with tile.TileContext(nc) as tc:
        with (
            tc.tile_pool(name="singles", bufs=1) as singles,
            tc.tile_pool(name="exq", bufs=36) as exq,
            tc.tile_pool(name="hcp", bufs=8) as hcpp,
            tc.tile_pool(name="outp", bufs=2) as outp,
            tc.tile_pool(name="ps_at", bufs=3, space="PSUM") as ps_at,
            tc.tile_pool(name="ps_o", bufs=2, space="PSUM") as ps_o,
            tc.tile_pool(name="ps_tr", bufs=1, space="PSUM") as ps_tr,
            tc.tile_pool(name="ps_mm", bufs=2, space="PSUM") as ps_mm,
        ):
            # ---------------- constants / singles ----------------
            # b_o enters the outproj PSUM accumulation as a rank-1 matmul
            # (ones-column x bo-row), so stores DMA straight from PSUM
            bo_sb = singles.tile([1, D], BF16, tag="bo", name="bo_sb")
            nc.gpsimd.dma_start(out=bo_sb, in_=ins["bo"])
            ones_col = singles.tile([1, P], BF16, tag="ones", name="ones_col")
            nc.gpsimd.memset(ones_col, 1.0)

            ident = singles.tile([P, P], BF16, tag="id", name="ident")
            make_identity(nc, ident)

            qt = singles.tile([P, PO, SC], BF16, tag="qt", name="qt")
            kt = singles.tile([P, PO, SC], BF16, tag="kt", name="kt")
            kt2 = singles.tile([P, PO, SC], BF16, tag="kt2", name="kt2")
            hcatT = singles.tile([P, PO, SC], BF16, tag="hct", name="hcatT")
            # V per head: [t', j', 64 V cols + ones col] -> denominator rides
            # the A@V matmul on free idx 64
            v_sb = [singles.tile([P, 16, 65], BF16, tag=f"v{hl}",
                                 name=f"v_sb{hl}") for hl in range(HPC)]
            for hl in range(HPC):
                nc.gpsimd.memset(v_sb[hl][:, :, 64:65], 1.0)

            # warmup fodder (keeps the PE p-state ramp warm during DMA waits)
            junk = singles.tile([P, 512], BF16, tag="junk", name="junk")
            nc.gpsimd.memset(junk, 0.0)

            # ---------------- input DMA emissions ----------------
            # Emission order == DMA service order (one pooled DMA device in
            # the cost model); order the critical first-scores chain first.
            xs = {}   # (name, s-half) -> tile  [P, KD, 256]
            ws = {}   # name -> tile [P, KD, D], loaded in f-halves
            xsrc = {"xq": ins["xqt"], "xk": ins["xkt"], "xv": ins["xvt"]}
            wsrc = {"wq": ins["wqt"], "wk": ins["wkt"], "wv": ins["wvt"],
                    "wo": ins["wot"]}
            for n, ap in wsrc.items():
                ws[n] = singles.tile([P, KD, D], BF16, tag=n, name=n)
            for n in xsrc:
                for sh in range(2):
                    xs[n, sh] = singles.tile([P, KD, 256], BF16,
                                             tag=f"{n}{sh}", name=f"{n}{sh}")

            def load_w(n, fh, quarter=False):
                src = wsrc[n].rearrange("(kd p) f -> p kd f", p=P)
                w = 256 if quarter else 512
                fs = slice(fh * w, (fh + 1) * w)
                nc.sync.dma_start(ws[n][:, :, fs], src[:, :, fs])

            def load_x(n, sh):
                src = xsrc[n].rearrange("(kd p) s -> p kd s", p=P)
                ss = slice(sh * 256, (sh + 1) * 256)
                nc.sync.dma_start(xs[n, sh], src[:, :, ss])

            def load_kt2(mlo, mhi, sh):
                """Grouped partition-rotated KT copy (kt2[p]=kt[(p+64)%128]).
                Placed in the SP DMA stream: its kt-copy waits block SP's
                in-order SEQ, which both throttles later (non-urgent) input
                loads and guarantees kt2 isn't starved on the DMA device."""
                ms = slice(mlo, mhi)
                ss = slice(sh * 256, (sh + 1) * 256)
                nc.sync.dma_start(kt2[0:64, ms, ss], kt[64:128, ms, ss])
                nc.sync.dma_start(kt2[64:128, ms, ss], kt[0:64, ms, ss])

            # critical chain first: first scores need wq-h0 + xq-h0 (Q mf0-3)
            # + wk quarters streaming. All remaining loads are emitted inside
            # quarter 0 (after the kt2-sh0 rotation DMAs) so the kt2 copies
            # get SP-queue priority over the long-slack input loads. kt2
            # emissions must FOLLOW their kt-copy producers (Tile deps track
            # emission order).
            load_w("wq", 0, True); load_x("xq", 0)
            load_x("xk", 0)
            load_w("wk", 0, True)
            load_w("wq", 1, True)
            load_w("wk", 1, True); load_w("wk", 2, True)
            load_w("wk", 3, True)

            def load_rest():
                load_w("wq", 1)
                load_w("wv", 0); load_x("xv", 0); load_w("wv", 1)
                load_x("xq", 1); load_x("xk", 1); load_x("xv", 1)
                load_w("wo", 0); load_w("wo", 1)

            # ---------------- PE warmup ----------------
            for _ in range(N_WARMUP):
                d_ps = ps_at.tile([P, 512], F32, tag="at", name="d_ps")
                nc.tensor.matmul(d_ps, junk[:, 0:P], junk, start=True,
                                 stop=True)

            # ---------------- projection units ----------------
            # proj_qk(t, mf, sh): QT/KT[f-chunk mf, s-half sh] = 8 kd matmuls
            # (free 256) + DVE copy; K also emits the kt2 rotated copy (SWDGE).
            def proj_qk(which, mf, sh):
                w_t = ws["wq" if which == "q" else "wk"]
                x_t = xs[("xq" if which == "q" else "xk"), sh]
                dst = qt if which == "q" else kt
                ss = slice(sh * 256, (sh + 1) * 256)
                ps = ps_mm.tile([P, 512], F32, tag="mm", name="ps")
                for kd in range(KD):
                    nc.tensor.matmul(
                        ps[:, 0:256], w_t[:, kd, mf * P:(mf + 1) * P],
                        x_t[:, kd, :], start=(kd == 0), stop=(kd == KD - 1))
                nc.vector.tensor_copy(dst[:, mf, ss], ps[:, 0:256])

            # V proj: V[s, f] for head hl, f-half nf (j' chunks 8nf..8nf+7)
            def v_proj(hl, nf):
                sh, so = hl // 2, (hl % 2) * 128
                ps = ps_mm.tile([P, 512], F32, tag="mm", name="ps")
                for kd in range(KD):
                    nc.tensor.matmul(
                        ps, xs["xv", sh][:, kd, so:so + P],
                        ws["wv"][:, kd, nf * 512:(nf + 1) * 512],
                        start=(kd == 0), stop=(kd == KD - 1))
                nc.vector.tensor_copy(
                    v_sb[hl][:, nf * 8:(nf + 1) * 8, 0:64],
                    ps.rearrange("p (j k) -> p j k", k=64))

            # ---------------- attention pieces ----------------
            # per (head, quarter qi): quarter = (pp, a); j'-order: same-parity
            # first (kt source) then opposite (kt2, which lands ~2us later).
            def jorder(a):
                return [j for j in range(16) if j % 2 == a] + \
                       [j for j in range(16) if j % 2 != a]

            ex_tiles = {}  # (hl, qi, j') -> SBUF bf16 [128, 512]

            def scores_exp(hl, qi, jp):
                pp, a = qi // 2, qi % 2
                hs = slice(hl * P, (hl + 1) * P)
                pb = slice(64 * a, 64 * a + 64)
                src = kt if jp % 2 == a else kt2
                at = ps_at.tile([P, 512], F32, tag="at", name="at")
                nc.tensor.matmul(at, src[pb, jp // 2, hs],
                                 qt[pb, 4 * pp:4 * pp + 4, hs],
                                 start=True, stop=True)
                ex = exq.tile([P, 512], BF16, tag="ex", name="ex")
                nc.scalar.activation(ex, at, EXPF, scale=0.125)
                ex_tiles[hl, qi, jp] = ex

            o_tiles = {}  # (hl, qi) -> PSUM [128, 4, 128]

            def av(hl, qi, jp, first, last):
                # one accumulation group per o4 bank: start zeroes the whole
                # 2KB zero-region lazily, so only the very first matmul may
                # set start and only the very last sets stop
                ex = ex_tiles.pop((hl, qi, jp))
                if first:
                    o_tiles[hl, qi] = ps_o.tile([P, 4, P], F32, tag="o",
                                                name="o4")
                o4 = o_tiles[hl, qi]
                for c in range(4):
                    nc.tensor.matmul(o4[:, c, 0:65],
                                     ex[:, c * P:(c + 1) * P],
                                     v_sb[hl][:, jp, :],
                                     start=first and c == 0,
                                     stop=last and c == 3)

            hcp_tiles = {}  # (hl, po) -> SBUF bf16 [128, 2, 64]

            def norm(hl, qi):
                """recip + 4 per-partition scaling multiplies for quarter qi."""
                pp, a = qi // 2, qi % 2
                o4 = o_tiles.pop((hl, qi))
                nc.vector.reciprocal(o4[:, :, 64:65], o4[:, :, 64:65])
                for c in range(4):
                    po = 4 * pp + c
                    if (hl, po) not in hcp_tiles:
                        hcp_tiles[hl, po] = hcpp.tile([P, 2, 64], BF16,
                                                      tag="hcp", name="hcp")
                    nc.vector.tensor_scalar(
                        hcp_tiles[hl, po][:, a, :], o4[:, c, 0:64],
                        o4[:, c, 64:65], None, MULT)

            def transposes(hl, pp):
                hs = slice(hl * P, (hl + 1) * P)
                tps = ps_tr.tile([P, 4, P], BF16, tag="tr", name="tps")
                for c in range(4):
                    po = 4 * pp + c
                    hcp = hcp_tiles.pop((hl, po))
                    nc.tensor.transpose(tps[:, c, :],
                                        hcp.rearrange("p a d -> p (a d)"),
                                        ident)
                    nc.vector.tensor_copy(hcatT[:, po, hs], tps[:, c, :])

            op_ps = {}

            def outproj(hl, nf, poh):
                """Half of the po-contraction for (head, f-half nf)."""
                hs = slice(hl * P, (hl + 1) * P)
                fs = slice(nf * 512, (nf + 1) * 512)
                if poh == 0:
                    ps = ps_mm.tile([P, 512], F32, tag="mm", name="ps")
                    op_ps[hl, nf] = ps
                    # bias via rank-1 matmul opens the accumulation group
                    nc.tensor.matmul(ps, ones_col, bo_sb[:, fs],
                                     start=True, stop=False)
                ps = op_ps[hl, nf]
                for po in range(4 * poh, 4 * poh + 4):
                    nc.tensor.matmul(ps, hcatT[:, po, hs], ws["wo"][:, po, fs],
                                     start=False, stop=(po == PO - 1))
                if poh == 1:
                    del op_ps[hl, nf]
                    os_t = outp.tile([P, 512], F32, tag="os", name="os_t")
                    if hl == HPC - 1:
                        # tail: ACT is idle after the last exp
                        nc.scalar.copy(os_t, ps)
                    else:
                        nc.vector.tensor_copy(os_t, ps)
                    nc.sync.dma_start(out_ap[hs, fs], os_t)

            # ---------------- schedule ----------------
            # Pre-attention: Q mf0-3 + K mf0-3 (s-half 0); quarter 0 runs a
            # custom j'-order with K mf4-7 interleaved as their DMA halves
            # land. All other proj / v_proj / outproj work becomes filler
            # units drained inside the attention loop at fixed slots — PE
            # runs ~2x faster than ACT during attention, so fillers absorb
            # PE idle gaps without starving ACT (3 at-banks = ~1.8us of
            # buffered ACT work rides out short PE stalls).
            # AVs lag scores by TWO quarters so every ex tile and V half is
            # long since ready when its AV executes.
            proj_qk("q", 0, 0)
            proj_qk("q", 1, 0)
            proj_qk("k", 0, 0)
            proj_qk("q", 2, 0)
            proj_qk("q", 3, 0)

# Deferred work units, drained at fixed slots inside the
            # attention loop (PE outruns ACT there, so units absorb PE idle
            # gaps). Every unit is keyed and ensure()d right before its
            # first consumer is emitted — slot arithmetic is a performance
            # heuristic, never a correctness requirement.
            from collections import OrderedDict
            units = OrderedDict()

            def kp_sh1(mf):
                proj_qk("k", mf, 1)
                if mf == 3:
                    load_kt2(0, 4, 1)
                elif mf == 7:
                    load_kt2(4, 8, 1)

            for mf in range(4, PO):
                units["qp", mf, 0] = lambda mf=mf: proj_qk("q", mf, 0)
            units["vp", 0, 0] = lambda: v_proj(0, 0)
            units["vp", 0, 1] = lambda: v_proj(0, 1)
            units["vp", 1, 0] = lambda: v_proj(1, 0)
            units["vp", 1, 1] = lambda: v_proj(1, 1)
            for mf in range(PO):
                units["kp", mf, 1] = lambda mf=mf: kp_sh1(mf)
                units["qp", mf, 1] = lambda mf=mf: proj_qk("q", mf, 1)
                if mf == 2:
                    units["vp", 2, 0] = lambda: v_proj(2, 0)
                    units["vp", 2, 1] = lambda: v_proj(2, 1)
                if mf == 4:
                    units["vp", 3, 0] = lambda: v_proj(3, 0)
                    units["vp", 3, 1] = lambda: v_proj(3, 1)

            def ensure(*key):
                fn = units.pop(key, None)
                if fn is not None:
                    fn()

            def fill(n=1):
                for _ in range(n):
                    if units:
                        units.pop(next(iter(units)))()

            def ensure_scores_deps(shl, sqi):
                """Emit any still-pending proj units whose outputs quarter
                (shl, sqi)'s scores read."""
                sh = shl // 2
                if sh == 1:
                    for mf in range(PO):
                        ensure("kp", mf, 1)
                mlo = 4 * (sqi // 2)
                for mf in range(mlo, mlo + 4):
                    ensure("qp", mf, sh)

            quarters = [(hl, qi) for hl in range(HPC) for qi in range(4)]
            jo_used = {}

            def post_av_block(ahl, aqi):
                norm(ahl, aqi)
                if aqi % 2 == 1:
                    transposes(ahl, aqi // 2)
                    if aqi == 3:
                        for nf, poh in ((0, 0), (0, 1), (1, 0), (1, 1)):
                            units["op", ahl, nf, poh] = (
                                lambda hl=ahl, nf=nf, poh=poh:
                                outproj(hl, nf, poh))

            for i, (hl, qi) in enumerate(quarters):
                avt = quarters[i - 2] if i >= 2 else None
                jo_av = jo_used[avt] if avt else None
                if i == 0:
                    # custom: quarter 0's kt-parity (even) j' interleaved
                    # with (a) K mf1-7 projections as the wk quarters land
                    # and (b) quarter 1's kt-parity (odd) j' — the K-proj
                    # phase is PE-bound at ~1.06us/score, so doubling the
                    # score stream here keeps ACT saturated. kt2 rotation
                    # DMAs then have a half-quarter of slack before the
                    # first kt2-parity j' needs them.
                    evens = [0, 2, 4, 6, 8, 10, 12, 14]
                    odds = [1, 3, 5, 7, 9, 11, 13, 15]
                    for idx in range(8):
                        scores_exp(hl, 0, evens[idx])
                        if idx < 7:
                            proj_qk("k", idx + 1, 0)
                        if idx == 2:
                            load_kt2(0, 4, 0)
                        elif idx == 6:
                            load_kt2(4, 8, 0)
                        elif idx == 7:
                            load_rest()
                        scores_exp(hl, 1, odds[idx])
                    for jp in odds:
                        scores_exp(hl, 0, jp)
                    jo_used[hl, 0] = evens + odds
                elif i == 1:
                    for idx, jp in enumerate(evens):
                        scores_exp(hl, 1, jp)
                        if idx % 2 == 1:
                            fill()
                    jo_used[hl, 1] = odds + evens
                else:
                    ensure_scores_deps(hl, qi)
                    if avt is not None and avt[1] == 0:
                        ensure("vp", avt[0], 0)
                        ensure("vp", avt[0], 1)
                    jo = jorder(qi % 2)
                    jo_used[hl, qi] = jo
                    for idx, jp in enumerate(jo):
                        scores_exp(hl, qi, jp)
                        if avt is not None:
                            av(*avt, jo_av[idx], idx == 0, idx == 15)
                        if idx % 3 == 2:
                            fill()
                if avt is not None:
                    post_av_block(*avt)

            # ---- tail ----
            # quarter (3,2): AVs + norm; head-3's po0-3 outproj partials can
            # already run (transposes(3,0) done after quarter (3,1)).
            thl, tqi = quarters[-2]
            jo_av = jo_used[thl, tqi]
            for idx, jp in enumerate(jo_av):
                av(thl, tqi, jp, idx == 0, idx == 15)
                fill()
            norm(thl, tqi)
            fill(len(units))
            outproj(thl, 0, 0)
            outproj(thl, 1, 0)

            # quarter (3,3): AVs, then a per-chunk pipelined finale so the
            # post-last-exp critical chain is as short as possible:
            # recip_c -> norm_c -> transpose_c -> copy_c -> outproj(po4+c),
            # then bias + store per f-half.
            thl, tqi = quarters[-1]
            jo_av = jo_used[thl, tqi]
            for idx, jp in enumerate(jo_av):
                av(thl, tqi, jp, idx == 0, idx == 15)
            o4 = o_tiles.pop((thl, tqi))
            hs = slice(thl * P, (thl + 1) * P)
            tps = ps_tr.tile([P, 4, P], BF16, tag="tr", name="tps")
            for c in range(4):
                po = 4 + c
                nc.vector.reciprocal(o4[:, c, 64:65], o4[:, c, 64:65])
                hcp = hcp_tiles.pop((thl, po))
                nc.vector.tensor_scalar(hcp[:, 1, :], o4[:, c, 0:64],
                                        o4[:, c, 64:65], None, MULT)
                nc.tensor.transpose(tps[:, c, :],
                                    hcp.rearrange("p a d -> p (a d)"), ident)
                nc.vector.tensor_copy(hcatT[:, po, hs], tps[:, c, :])
                for nf in range(2):
                    fs = slice(nf * 512, (nf + 1) * 512)
                    nc.tensor.matmul(op_ps[thl, nf], hcatT[:, po, hs],
                                     ws["wo"][:, po, fs],
                                     start=False, stop=(po == PO - 1))
            for nf in range(2):
                fs = slice(nf * 512, (nf + 1) * 512)
                ps = op_ps.pop((thl, nf))
                os_t = outp.tile([P, 512], F32, tag="os", name="os_t")
                nc.scalar.copy(os_t, ps)
                nc.sync.dma_start(out_ap[hs, fs], os_t)
    return nc


def build_program():
    nc = bacc.Bacc("TRN2", target_bir_lowering=False, debug=False,
                   enable_asserts=False, num_devices=NCORES)
    ins = {
        "xqt": nc.dram_tensor("xqt", [D, SC], BF16, kind="ExternalInput").ap(),
        "xkt": nc.dram_tensor("xkt", [D, SC], BF16, kind="ExternalInput").ap(),
        "xvt": nc.dram_tensor("xvt", [D, SC], BF16, kind="ExternalInput").ap(),
        "wqt": nc.dram_tensor("wqt", [D, D], BF16, kind="ExternalInput").ap(),
        "wkt": nc.dram_tensor("wkt", [D, D], BF16, kind="ExternalInput").ap(),
        "wvt": nc.dram_tensor("wvt", [D, D], BF16, kind="ExternalInput").ap(),
        "wot": nc.dram_tensor("wot", [D, D], BF16, kind="ExternalInput").ap(),
        "bo": nc.dram_tensor("bo", [1, D], BF16, kind="ExternalInput").ap(),
    }
    out_ap = nc.dram_tensor("out", [SC, D], F32, kind="ExternalOutput").ap()
    build_body(nc, out_ap, ins)
    nc.finalize()
    return nc


def make_in_maps(inputs):
    import ml_dtypes
    ndt = ml_dtypes.bfloat16
    Xq = np.asarray(inputs["X_q"], dtype=np.float32)
    Xk = np.asarray(inputs["X_k"], dtype=np.float32)
    Xv = np.asarray(inputs["X_v"], dtype=np.float32)
    wqt = np.ascontiguousarray(np.asarray(inputs["W_q"], np.float32).T).astype(ndt)
    wkt = np.ascontiguousarray(np.asarray(inputs["W_k"], np.float32).T).astype(ndt)
    wvt = np.ascontiguousarray(np.asarray(inputs["W_v"], np.float32).T).astype(ndt)
    wot = np.ascontiguousarray(np.asarray(inputs["W_o"], np.float32).T).astype(ndt)
    bo = np.asarray(inputs["b_o"], np.float32).reshape(1, D).astype(ndt)
    xt = {n: [np.ascontiguousarray(x[b].T).astype(ndt) for b in range(B)]
          for n, x in (("xqt", Xq), ("xkt", Xk), ("xvt", Xv))}
    in_maps = []
    for c in range(NCORES):
        b, g = divmod(c, NCORES // B)
        sl = slice(g * SC, (g + 1) * SC)
        in_maps.append({
            "xqt": np.ascontiguousarray(xt["xqt"][b][:, sl]),
            "xkt": np.ascontiguousarray(xt["xkt"][b][:, sl]),
            "xvt": np.ascontiguousarray(xt["xvt"][b][:, sl]),
            "wqt": wqt, "wkt": wkt, "wvt": wvt, "wot": wot, "bo": bo,
        })
    return in_maps


_NC_CACHE = {}


def _run(inputs, trace=False, trace_cores=None):
    from concourse.bass_utils import run_bass_kernel_spmd
    if MODE not in _NC_CACHE:
        _NC_CACHE[MODE] = build_program()
    nc = _NC_CACHE[MODE]
    in_maps = make_in_maps(inputs)
    res = run_bass_kernel_spmd(nc, in_maps, core_ids=list(range(NCORES)),
                               trace=trace, trace_cores=trace_cores)
    out = np.empty((B, S, D), dtype=np.float32)
    for c in range(NCORES):
        b, g = divmod(c, NCORES // B)
        out[b, g * SC:(g + 1) * SC, :] = res.results[c]["out"]
    return out, res


def kernel(**inputs):
    out, _ = _run(inputs, trace=False)
    return out


# revision 51
# speedup vs baseline: 1.2338x; 1.0203x over previous
"""MultiHeadAttention (faithful raw-reshape variant) on 8 trn2 NeuronCores.

Math (per batch b):
  Y  = Xq @ Wq.T            [S, D]
  Z  = Xk @ Wk.T            [S, D]
  V  = Xv @ Wv.T            [S, D]
  reshape (B,S,D)->(B,H,S,dk) is a *raw view*: head h <- rows [128h, 128h+128)
  of Y/Z/V; within the block, q = 16t + j maps to (row t, features 64j..64j+64).
  A  = softmax(Qh @ Kh.T / 8), O = A @ Vh, placed back into the same raw view,
  out = Hcat @ Wo.T + b_o.

Heads partition the *rows* of Y/Z/V, so work is fully independent across
(b, h): 32 tasks, 4 per core, no collectives.

Per-core device program (heads hl=0..3 over the core's 512 rows):
  QT/KT: transposed projections QT[f, s] ([128, 8, 512]); kt2 = partition-
         rotated KT copy so either j'-parity is on either partition half.
  scores: at[t', 128c+t] = sum_d K[(t',j'), d] Q[(t,j(c)), d]; one matmul per
         (head, quarter, j'): stationary kt-chunk [64, 128], moving qt [64,512].
  exp:   ACT, scale=1/8 fused, fp32 PSUM -> bf16 SBUF. ACT is the bottleneck
         engine (256 x ~0.6us); the schedule keeps it saturated.
  AV:    O[q, f] orientation: stationary ex-chunk [128, 128], moving
         V-chunk [128, 65] (64 V cols + ones col -> softmax denominator on
         free idx 64). 16 j'-chunks accumulate per PSUM tile [128, 4, 128].
         AVs lag scores by one quarter so PE never waits on ACT results.
  norm:  DVE reciprocal of denom cols + per-partition tensor_scalar multiply
         into bf16 hcp tiles [128 t, 2 side, 64 d] (= Hcat f-chunks).
  transpose: PE transpose (via identity) hcp -> hcatT chunks [128 f, 128 t].
  out:   out[t, g] = hcatT.T @ WoT + b_o; stores ride the DVE queue.
"""

import os

import numpy as np

import concourse.bass as bass
import concourse.mybir as mybir
import concourse.tile as tile
from concourse import bacc
from concourse.masks import make_identity

B, S, D = 2, 2048, 1024
H, DK = 16, 64
NCORES = 8
HPC = H // (NCORES // B)  # heads per core = 4
SC = HPC * 128            # s-rows per core = 512
P = 128
KD = D // P               # 8 contraction chunks
PO = D // P               # 8 feature chunks
F32 = mybir.dt.float32
BF16 = mybir.dt.bfloat16

MODE = "bf16"
N_WARMUP = int(os.environ.get("TRN_WARMUP", "14"))


def build_body(nc, out_ap, ins):
    """Emit the per-core program. ins: dict of DRAM APs."""
    EXPF = mybir.ActivationFunctionType.Exp
    MULT = mybir.AluOpType.mult
    ADD = mybir.AluOpType.add

    with tile.TileContext(nc) as tc:
        with (
            tc.tile_pool(name="singles", bufs=1) as singles,
            tc.tile_pool(name="exq", bufs=36) as exq,
            tc.tile_pool(name="hcp", bufs=8) as hcpp,
            tc.tile_pool(name="outp", bufs=2) as outp,
            tc.tile_pool(name="ps_at", bufs=3, space="PSUM") as ps_at,
            tc.tile_pool(name="ps_o", bufs=2, space="PSUM") as ps_o,
            tc.tile_pool(name="ps_tr", bufs=1, space="PSUM") as ps_tr,
            tc.tile_pool(name="ps_mm", bufs=2, space="PSUM") as ps_mm,
        ):
            # ---------------- constants / singles ----------------
            # b_o enters the outproj PSUM accumulation as a rank-1 matmul
            # (ones-column x bo-row), so stores DMA straight from PSUM
            bo_sb = singles.tile([1, D], BF16, tag="bo", name="bo_sb")
            nc.gpsimd.dma_start(out=bo_sb, in_=ins["bo"])
            ones_col = singles.tile([1, P], BF16, tag="ones", name="ones_col")
            nc.gpsimd.memset(ones_col, 1.0)

            ident = singles.tile([P, P], BF16, tag="id", name="ident")
            make_identity(nc, ident)

            qt = singles.tile([P, PO, SC], BF16, tag="qt", name="qt")
            kt = singles.tile([P, PO, SC], BF16, tag="kt", name="kt")
            kt2 = singles.tile([P, PO, SC], BF16, tag="kt2", name="kt2")
            hcatT = singles.tile([P, PO, SC], BF16, tag="hct", name="hcatT")
            # V per head: [t', j', 64 V cols + ones col] -> denominator rides
            # the A@V matmul on free idx 64
            v_sb = [singles.tile([P, 16, 65], BF16, tag=f"v{hl}",
                                 name=f"v_sb{hl}") for hl in range(HPC)]
            for hl in range(HPC):
                nc.gpsimd.memset(v_sb[hl][:, :, 64:65], 1.0)

            # warmup fodder (keeps the PE p-state ramp warm during DMA waits)
            junk = singles.tile([P, 512], BF16, tag="junk", name="junk")
            nc.gpsimd.memset(junk, 0.0)

            # ---------------- input DMA emissions ----------------
            # Emission order == DMA service order (one pooled DMA device in
            # the cost model); order the critical first-scores chain first.
            xs = {}   # (name, s-half) -> tile  [P, KD, 256]
            ws = {}   # name -> tile [P, KD, D], loaded in f-halves
            xsrc = {"xq": ins["xqt"], "xk": ins["xkt"], "xv": ins["xvt"]}
            wsrc = {"wq": ins["wqt"], "wk": ins["wkt"], "wv": ins["wvt"],
                    "wo": ins["wot"]}
            for n, ap in wsrc.items():
                ws[n] = singles.tile([P, KD, D], BF16, tag=n, name=n)
            for n in xsrc:
                for sh in range(2):
                    xs[n, sh] = singles.tile([P, KD, 256], BF16,
                                             tag=f"{n}{sh}", name=f"{n}{sh}")

            def load_w(n, fh, quarter=False):
                src = wsrc[n].rearrange("(kd p) f -> p kd f", p=P)
                w = 256 if quarter else 512
                fs = slice(fh * w, (fh + 1) * w)
                nc.sync.dma_start(ws[n][:, :, fs], src[:, :, fs])

            def load_x(n, sh):
                src = xsrc[n].rearrange("(kd p) s -> p kd s", p=P)
                ss = slice(sh * 256, (sh + 1) * 256)
                nc.sync.dma_start(xs[n, sh], src[:, :, ss])

            def load_kt2(mlo, mhi, sh):
                """Grouped partition-rotated KT copy (kt2[p]=kt[(p+64)%128]).
                Placed in the SP DMA stream: its kt-copy waits block SP's
                in-order SEQ, which both throttles later (non-urgent) input
                loads and guarantees kt2 isn't starved on the DMA device."""
                ms = slice(mlo, mhi)
                ss = slice(sh * 256, (sh + 1) * 256)
                nc.sync.dma_start(kt2[0:64, ms, ss], kt[64:128, ms, ss])
                nc.sync.dma_start(kt2[64:128, ms, ss], kt[0:64, ms, ss])

            # critical chain first: first scores need wq-h0 + xq-h0 (Q mf0-3)
            # + wk quarters streaming. All remaining loads are emitted inside
            # quarter 0 (after the kt2-sh0 rotation DMAs) so the kt2 copies
            # get SP-queue priority over the long-slack input loads. kt2
            # emissions must FOLLOW their kt-copy producers (Tile deps track
            # emission order).
            load_w("wq", 0, True); load_x("xq", 0)
            load_x("xk", 0)
            load_w("wk", 0, True)
            load_w("wq", 1, True)
            load_w("wk", 1, True); load_w("wk", 2, True)
            load_w("wk", 3, True)

            def load_rest():
                load_w("wq", 1)
                load_w("wv", 0); load_x("xv", 0); load_w("wv", 1)
                load_x("xq", 1); load_x("xk", 1); load_x("xv", 1)
                load_w("wo", 0); load_w("wo", 1)

            # ---------------- PE warmup ----------------
            for _ in range(N_WARMUP):
                d_ps = ps_at.tile([P, 512], F32, tag="at", name="d_ps")
                nc.tensor.matmul(d_ps, junk[:, 0:P], junk, start=True,
                                 stop=True)

            # ---------------- projection units ----------------
            # proj_qk(t, mf, sh): QT/KT[f-chunk mf, s-half sh] = 8 kd matmuls
            # (free 256) + DVE copy. `part` splits the kd accumulation into
            # two 4-matmul drains so a unit never blocks score emission for
            # more than ~0.4us of PE time (ACT buffer is only ~1.8us deep).
            proj_ps = {}

            def proj_qk(which, mf, sh, part=None):
                w_t = ws["wq" if which == "q" else "wk"]
                x_t = xs[("xq" if which == "q" else "xk"), sh]
                dst = qt if which == "q" else kt
                ss = slice(sh * 256, (sh + 1) * 256)
                kds = range(KD) if part is None else \
                    range(part * 4, part * 4 + 4)
                if part in (None, 0):
                    proj_ps[which, mf, sh] = ps_mm.tile([P, 512], F32,
                                                        tag="mm", name="ps")
                ps = proj_ps[which, mf, sh]
                for kd in kds:
                    nc.tensor.matmul(
                        ps[:, 0:256], w_t[:, kd, mf * P:(mf + 1) * P],
                        x_t[:, kd, :], start=(kd == 0), stop=(kd == KD - 1))
                if part in (None, 1):
                    del proj_ps[which, mf, sh]
                    nc.vector.tensor_copy(dst[:, mf, ss], ps[:, 0:256])

            # V proj: V[s, f] for head hl, f-half nf (j' chunks 8nf..8nf+7)
            def v_proj(hl, nf, part=None):
                sh, so = hl // 2, (hl % 2) * 128
                kds = range(KD) if part is None else \
                    range(part * 4, part * 4 + 4)
                if part in (None, 0):
                    proj_ps["v", hl, nf] = ps_mm.tile([P, 512], F32,
                                                      tag="mm", name="ps")
                ps = proj_ps["v", hl, nf]
                for kd in kds:
                    nc.tensor.matmul(
                        ps, xs["xv", sh][:, kd, so:so + P],
                        ws["wv"][:, kd, nf * 512:(nf + 1) * 512],
                        start=(kd == 0), stop=(kd == KD - 1))
                if part in (None, 1):
                    del proj_ps["v", hl, nf]
                    nc.vector.tensor_copy(
                        v_sb[hl][:, nf * 8:(nf + 1) * 8, 0:64],
                        ps.rearrange("p (j k) -> p j k", k=64))

            # ---------------- attention pieces ----------------
            # per (head, quarter qi): quarter = (pp, a); j'-order: same-parity
            # first (kt source) then opposite (kt2, which lands ~2us later).
            def jorder(a):
                return [j for j in range(16) if j % 2 == a] + \
                       [j for j in range(16) if j % 2 != a]

            ex_tiles = {}  # (hl, qi, j') -> SBUF bf16 [128, 512] (or半 pair)

            def scores_exp(hl, qi, jp, half=None):
                """half=0/1: free-256 score tile covering po-chunk pair
                (4pp+2*half) — lets the first exps start before Q mf2-3
                project. ex_tiles holds a (loA, hiB) tuple in that case."""
                pp, a = qi // 2, qi % 2
                hs = slice(hl * P, (hl + 1) * P)
                pb = slice(64 * a, 64 * a + 64)
                src = kt if jp % 2 == a else kt2
                at = ps_at.tile([P, 512], F32, tag="at", name="at")
                if half is None:
                    po_s = slice(4 * pp, 4 * pp + 4)
                    at_v, wf = at, 512
                else:
                    po_s = slice(4 * pp + 2 * half, 4 * pp + 2 * half + 2)
                    at_v, wf = at[:, 0:256], 256
                nc.tensor.matmul(at_v, src[pb, jp // 2, hs],
                                 qt[pb, po_s, hs], start=True, stop=True)
                ex = exq.tile([P, 512], BF16, tag="ex", name="ex")
                nc.scalar.activation(ex[:, 0:wf], at_v, EXPF, scale=0.125)
                if half is None:
                    ex_tiles[hl, qi, jp] = ex
                else:
                    pair = ex_tiles.setdefault((hl, qi, jp), [None, None])
                    pair[half] = ex

            def ex_chunk(exv, c):
                if isinstance(exv, list):
                    return exv[c // 2][:, (c % 2) * P:(c % 2 + 1) * P]
                return exv[:, c * P:(c + 1) * P]

            o_tiles = {}  # (hl, qi) -> PSUM [128, 4, 128]

            def av(hl, qi, jp, first, last):
                # one accumulation group per o4 bank: start zeroes the whole
                # 2KB zero-region lazily, so only the very first matmul may
                # set start and only the very last sets stop
                exv = ex_tiles.pop((hl, qi, jp))
                if first:
                    o_tiles[hl, qi] = ps_o.tile([P, 4, P], F32, tag="o",
                                                name="o4")
                o4 = o_tiles[hl, qi]
                for c in range(4):
                    nc.tensor.matmul(o4[:, c, 0:65], ex_chunk(exv, c),
                                     v_sb[hl][:, jp, :],
                                     start=first and c == 0,
                                     stop=last and c == 3)

            hcp_tiles = {}  # (hl, po) -> SBUF bf16 [128, 2, 64]

            def norm(hl, qi):
                """recip + 4 per-partition scaling multiplies for quarter qi."""
                pp, a = qi // 2, qi % 2
                o4 = o_tiles.pop((hl, qi))
                nc.vector.reciprocal(o4[:, :, 64:65], o4[:, :, 64:65])
                for c in range(4):
                    po = 4 * pp + c
                    if (hl, po) not in hcp_tiles:
                        hcp_tiles[hl, po] = hcpp.tile([P, 2, 64], BF16,
                                                      tag="hcp", name="hcp")
                    nc.vector.tensor_scalar(
                        hcp_tiles[hl, po][:, a, :], o4[:, c, 0:64],
                        o4[:, c, 64:65], None, MULT)

            def transposes(hl, pp):
                hs = slice(hl * P, (hl + 1) * P)
                tps = ps_tr.tile([P, 4, P], BF16, tag="tr", name="tps")
                for c in range(4):
                    po = 4 * pp + c
                    hcp = hcp_tiles.pop((hl, po))
                    nc.tensor.transpose(tps[:, c, :],
                                        hcp.rearrange("p a d -> p (a d)"),
                                        ident)
                    nc.vector.tensor_copy(hcatT[:, po, hs], tps[:, c, :])

            op_ps = {}

            def outproj(hl, nf, poh):
                """Half of the po-contraction for (head, f-half nf)."""
                hs = slice(hl * P, (hl + 1) * P)
                fs = slice(nf * 512, (nf + 1) * 512)
                if poh == 0:
                    ps = ps_mm.tile([P, 512], F32, tag="mm", name="ps")
                    op_ps[hl, nf] = ps
                    # bias via rank-1 matmul opens the accumulation group
                    nc.tensor.matmul(ps, ones_col, bo_sb[:, fs],
                                     start=True, stop=False)
                ps = op_ps[hl, nf]
                for po in range(4 * poh, 4 * poh + 4):
                    nc.tensor.matmul(ps, hcatT[:, po, hs], ws["wo"][:, po, fs],
                                     start=False, stop=(po == PO - 1))
                if poh == 1:
                    del op_ps[hl, nf]
                    os_t = outp.tile([P, 512], F32, tag="os", name="os_t")
                    if hl == HPC - 1:
                        # tail: ACT is idle after the last exp
                        nc.scalar.copy(os_t, ps)
                    else:
                        nc.vector.tensor_copy(os_t, ps)
                    nc.sync.dma_start(out_ap[hs, fs], os_t)

            # ---------------- schedule ----------------
            # Pre-attention: Q mf0-3 + K mf0-3 (s-half 0); quarter 0 runs a
            # custom j'-order with K mf4-7 interleaved as their DMA halves
            # land. All other proj / v_proj / outproj work becomes filler
            # units drained inside the attention loop at fixed slots — PE
            # runs ~2x faster than ACT during attention, so fillers absorb
            # PE idle gaps without starving ACT (3 at-banks = ~1.8us of
            # buffered ACT work rides out short PE stalls).
            # AVs lag scores by TWO quarters so every ex tile and V half is
            # long since ready when its AV executes.
            proj_qk("q", 0, 0)
            proj_qk("q", 1, 0)
            proj_qk("k", 0, 0)
            proj_qk("q", 2, 0)
            proj_qk("q", 3, 0)

# Deferred work units, drained at fixed slots inside the
            # attention loop (PE outruns ACT there, so units absorb PE idle
            # gaps). Every unit is keyed and ensure()d right before its
            # first consumer is emitted — slot arithmetic is a performance
            # heuristic, never a correctness requirement.
            from collections import OrderedDict
            units = OrderedDict()

            def kp_sh1_tail(mf):
                proj_qk("k", mf, 1, 1)
                if mf == 3:
                    load_kt2(0, 4, 1)
                elif mf == 7:
                    load_kt2(4, 8, 1)

            def add_unit(kind, a, b):
                if kind == "qp":
                    units["qp", a, b, 0] = lambda: proj_qk("q", a, b, 0)
                    units["qp", a, b, 1] = lambda: proj_qk("q", a, b, 1)
                elif kind == "kp":
                    units["kp", a, b, 0] = lambda: proj_qk("k", a, b, 0)
                    units["kp", a, b, 1] = lambda: kp_sh1_tail(a)
                else:
                    units["vp", a, b, 0] = lambda: v_proj(a, b, 0)
                    units["vp", a, b, 1] = lambda: v_proj(a, b, 1)

            for mf in range(4, PO):
                add_unit("qp", mf, 0)
            add_unit("vp", 0, 0)
            add_unit("vp", 0, 1)
            add_unit("vp", 1, 0)
            add_unit("vp", 1, 1)
            for mf in range(PO):
                add_unit("kp", mf, 1)
                add_unit("qp", mf, 1)
                if mf == 2:
                    add_unit("vp", 2, 0)
                    add_unit("vp", 2, 1)
                if mf == 4:
                    add_unit("vp", 3, 0)
                    add_unit("vp", 3, 1)

            def ensure(*key):
                fn = units.pop(key, None)
                if fn is not None:
                    fn()

            def ensure2(kind, a, b):
                ensure(kind, a, b, 0)
                ensure(kind, a, b, 1)

            def fill(n=1):
                for _ in range(n):
                    if units:
                        units.pop(next(iter(units)))()

            def ensure_scores_deps(shl, sqi):
                """Emit any still-pending proj units whose outputs quarter
                (shl, sqi)'s scores read."""
                sh = shl // 2
                if sh == 1:
                    for mf in range(PO):
                        ensure2("kp", mf, 1)
                mlo = 4 * (sqi // 2)
                for mf in range(mlo, mlo + 4):
                    ensure2("qp", mf, sh)

            quarters = [(hl, qi) for hl in range(HPC) for qi in range(4)]
            jo_used = {}

            def post_av_block(ahl, aqi):
                norm(ahl, aqi)
                if aqi % 2 == 1:
                    transposes(ahl, aqi // 2)
                    if aqi == 3:
                        for nf, poh in ((0, 0), (0, 1), (1, 0), (1, 1)):
                            units["op", ahl, nf, poh] = (
                                lambda hl=ahl, nf=nf, poh=poh:
                                outproj(hl, nf, poh))

            for i, (hl, qi) in enumerate(quarters):
                avt = quarters[i - 2] if i >= 2 else None
                jo_av = jo_used[avt] if avt else None
                if i == 0:
                    # custom: quarter 0's kt-parity (even) j' interleaved
                    # with (a) K mf1-7 projections as the wk quarters land
                    # and (b) quarter 1's kt-parity (odd) j' — the K-proj
                    # phase is PE-bound at ~1.06us/score, so doubling the
                    # score stream here keeps ACT saturated. kt2 rotation
                    # DMAs then have a half-quarter of slack before the
                    # first kt2-parity j' needs them.
                    evens = [0, 2, 4, 6, 8, 10, 12, 14]
                    odds = [1, 3, 5, 7, 9, 11, 13, 15]
                    for idx in range(8):
                        scores_exp(hl, 0, evens[idx])
                        if idx < 7:
                            proj_qk("k", idx + 1, 0)
                        if idx == 2:
                            load_kt2(0, 4, 0)
                        elif idx == 6:
                            load_kt2(4, 8, 0)
                        elif idx == 7:
                            load_rest()
                        scores_exp(hl, 1, odds[idx])
                    for jp in odds:
                        scores_exp(hl, 0, jp)
                    jo_used[hl, 0] = evens + odds
                elif i == 1:
                    for idx, jp in enumerate(evens):
                        scores_exp(hl, 1, jp)
                        if idx % 2 == 1:
                            fill()
                    jo_used[hl, 1] = odds + evens
                else:
                    ensure_scores_deps(hl, qi)
                    if avt is not None and avt[1] == 0:
                        ensure2("vp", avt[0], 0)
                        ensure2("vp", avt[0], 1)
                    jo = jorder(qi % 2)
                    jo_used[hl, qi] = jo
                    for idx, jp in enumerate(jo):
                        scores_exp(hl, qi, jp)
                        if avt is not None:
                            av(*avt, jo_av[idx], idx == 0, idx == 15)
                        if idx % 3 == 2:
                            fill()
                if avt is not None:
                    post_av_block(*avt)

            # ---- tail ----
            # quarter (3,2): AVs + norm; head-3's po0-3 outproj partials can
            # already run (transposes(3,0) done after quarter (3,1)).
            thl, tqi = quarters[-2]
            jo_av = jo_used[thl, tqi]
            for idx, jp in enumerate(jo_av):
                av(thl, tqi, jp, idx == 0, idx == 15)
                fill()
            norm(thl, tqi)
            fill(len(units))
            outproj(thl, 0, 0)
            outproj(thl, 1, 0)

            # quarter (3,3): AVs, then a per-chunk pipelined finale so the
            # post-last-exp critical chain is as short as possible:
            # recip_c -> norm_c -> transpose_c -> copy_c -> outproj(po4+c),
            # then bias + store per f-half.
            thl, tqi = quarters[-1]
            jo_av = jo_used[thl, tqi]
            for idx, jp in enumerate(jo_av):
                av(thl, tqi, jp, idx == 0, idx == 15)
            o4 = o_tiles.pop((thl, tqi))
            hs = slice(thl * P, (thl + 1) * P)
            tps = ps_tr.tile([P, 4, P], BF16, tag="tr", name="tps")
            for c in range(4):
                po = 4 + c
                nc.vector.reciprocal(o4[:, c, 64:65], o4[:, c, 64:65])
                hcp = hcp_tiles.pop((thl, po))
                nc.vector.tensor_scalar(hcp[:, 1, :], o4[:, c, 0:64],
                                        o4[:, c, 64:65], None, MULT)
                nc.tensor.transpose(tps[:, c, :],
                                    hcp.rearrange("p a d -> p (a d)"), ident)
                nc.vector.tensor_copy(hcatT[:, po, hs], tps[:, c, :])
                for nf in range(2):
                    fs = slice(nf * 512, (nf + 1) * 512)
                    nc.tensor.matmul(op_ps[thl, nf], hcatT[:, po, hs],
                                     ws["wo"][:, po, fs],
                                     start=False, stop=(po == PO - 1))
            for nf in range(2):
                fs = slice(nf * 512, (nf + 1) * 512)
                ps = op_ps.pop((thl, nf))
                os_t = outp.tile([P, 512], F32, tag="os", name="os_t")
                nc.scalar.copy(os_t, ps)
                nc.sync.dma_start(out_ap[hs, fs], os_t)
    return nc


def build_program():
    nc = bacc.Bacc("TRN2", target_bir_lowering=False, debug=False,
                   enable_asserts=False, num_devices=NCORES)
    ins = {
        "xqt": nc.dram_tensor("xqt", [D, SC], BF16, kind="ExternalInput").ap(),
        "xkt": nc.dram_tensor("xkt", [D, SC], BF16, kind="ExternalInput").ap(),
        "xvt": nc.dram_tensor("xvt", [D, SC], BF16, kind="ExternalInput").ap(),
        "wqt": nc.dram_tensor("wqt", [D, D], BF16, kind="ExternalInput").ap(),
        "wkt": nc.dram_tensor("wkt", [D, D], BF16, kind="ExternalInput").ap(),
        "wvt": nc.dram_tensor("wvt", [D, D], BF16, kind="ExternalInput").ap(),
        "wot": nc.dram_tensor("wot", [D, D], BF16, kind="ExternalInput").ap(),
        "bo": nc.dram_tensor("bo", [1, D], BF16, kind="ExternalInput").ap(),
    }
    out_ap = nc.dram_tensor("out", [SC, D], F32, kind="ExternalOutput").ap()
    build_body(nc, out_ap, ins)
    nc.finalize()
    return nc


def make_in_maps(inputs):
    import ml_dtypes
    ndt = ml_dtypes.bfloat16
    Xq = np.asarray(inputs["X_q"], dtype=np.float32)
    Xk = np.asarray(inputs["X_k"], dtype=np.float32)
    Xv = np.asarray(inputs["X_v"], dtype=np.float32)
    wqt = np.ascontiguousarray(np.asarray(inputs["W_q"], np.float32).T).astype(ndt)
    wkt = np.ascontiguousarray(np.asarray(inputs["W_k"], np.float32).T).astype(ndt)
    wvt = np.ascontiguousarray(np.asarray(inputs["W_v"], np.float32).T).astype(ndt)
    wot = np.ascontiguousarray(np.asarray(inputs["W_o"], np.float32).T).astype(ndt)
    bo = np.asarray(inputs["b_o"], np.float32).reshape(1, D).astype(ndt)
    xt = {n: [np.ascontiguousarray(x[b].T).astype(ndt) for b in range(B)]
          for n, x in (("xqt", Xq), ("xkt", Xk), ("xvt", Xv))}
    in_maps = []
    for c in range(NCORES):
        b, g = divmod(c, NCORES // B)
        sl = slice(g * SC, (g + 1) * SC)
        in_maps.append({
            "xqt": np.ascontiguousarray(xt["xqt"][b][:, sl]),
            "xkt": np.ascontiguousarray(xt["xkt"][b][:, sl]),
            "xvt": np.ascontiguousarray(xt["xvt"][b][:, sl]),
            "wqt": wqt, "wkt": wkt, "wvt": wvt, "wot": wot, "bo": bo,
        })
    return in_maps


_NC_CACHE = {}


def _run(inputs, trace=False, trace_cores=None):
    from concourse.bass_utils import run_bass_kernel_spmd
    if MODE not in _NC_CACHE:
        _NC_CACHE[MODE] = build_program()
    nc = _NC_CACHE[MODE]
    in_maps = make_in_maps(inputs)
    res = run_bass_kernel_spmd(nc, in_maps, core_ids=list(range(NCORES)),
                               trace=trace, trace_cores=trace_cores)
    out = np.empty((B, S, D), dtype=np.float32)
    for c in range(NCORES):
        b, g = divmod(c, NCORES // B)
        out[b, g * SC:(g + 1) * SC, :] = res.results[c]["out"]
    return out, res


def kernel(**inputs):
    out, _ = _run(inputs, trace=False)
    return out


# revision 52
# speedup vs baseline: 1.2339x; 1.0001x over previous
"""MultiHeadAttention (faithful raw-reshape variant) on 8 trn2 NeuronCores.

Math (per batch b):
  Y  = Xq @ Wq.T            [S, D]
  Z  = Xk @ Wk.T            [S, D]
  V  = Xv @ Wv.T            [S, D]
  reshape (B,S,D)->(B,H,S,dk) is a *raw view*: head h <- rows [128h, 128h+128)
  of Y/Z/V; within the block, q = 16t + j maps to (row t, features 64j..64j+64).
  A  = softmax(Qh @ Kh.T / 8), O = A @ Vh, placed back into the same raw view,
  out = Hcat @ Wo.T + b_o.

Heads partition the *rows* of Y/Z/V, so work is fully independent across
(b, h): 32 tasks, 4 per core, no collectives.

Per-core device program (heads hl=0..3 over the core's 512 rows):
  QT/KT: transposed projections QT[f, s] ([128, 8, 512]); kt2 = partition-
         rotated KT copy so either j'-parity is on either partition half.
  scores: at[t', 128c+t] = sum_d K[(t',j'), d] Q[(t,j(c)), d]; one matmul per
         (head, quarter, j'): stationary kt-chunk [64, 128], moving qt [64,512].
  exp:   ACT, scale=1/8 fused, fp32 PSUM -> bf16 SBUF. ACT is the bottleneck
         engine (256 x ~0.6us); the schedule keeps it saturated.
  AV:    O[q, f] orientation: stationary ex-chunk [128, 128], moving
         V-chunk [128, 65] (64 V cols + ones col -> softmax denominator on
         free idx 64). 16 j'-chunks accumulate per PSUM tile [128, 4, 128].
         AVs lag scores by one quarter so PE never waits on ACT results.
  norm:  DVE reciprocal of denom cols + per-partition tensor_scalar multiply
         into bf16 hcp tiles [128 t, 2 side, 64 d] (= Hcat f-chunks).
  transpose: PE transpose (via identity) hcp -> hcatT chunks [128 f, 128 t].
  out:   out[t, g] = hcatT.T @ WoT + b_o; stores ride the DVE queue.
"""

import os

import numpy as np

import concourse.bass as bass
import concourse.mybir as mybir
import concourse.tile as tile
from concourse import bacc
from concourse.masks import make_identity

B, S, D = 2, 2048, 1024
H, DK = 16, 64
NCORES = 8
HPC = H // (NCORES // B)  # heads per core = 4
SC = HPC * 128            # s-rows per core = 512
P = 128
KD = D // P               # 8 contraction chunks
PO = D // P               # 8 feature chunks
F32 = mybir.dt.float32
BF16 = mybir.dt.bfloat16

MODE = "bf16"
N_WARMUP = int(os.environ.get("TRN_WARMUP", "14"))


def build_body(nc, out_ap, ins):
    """Emit the per-core program. ins: dict of DRAM APs."""
    EXPF = mybir.ActivationFunctionType.Exp
    MULT = mybir.AluOpType.mult
    ADD = mybir.AluOpType.add

    with tile.TileContext(nc) as tc:
        with (
            tc.tile_pool(name="singles", bufs=1) as singles,
            tc.tile_pool(name="exq", bufs=36) as exq,
            tc.tile_pool(name="hcp", bufs=8) as hcpp,
            tc.tile_pool(name="outp", bufs=2) as outp,
            tc.tile_pool(name="ps_at", bufs=3, space="PSUM") as ps_at,
            tc.tile_pool(name="ps_o", bufs=2, space="PSUM") as ps_o,
            tc.tile_pool(name="ps_tr", bufs=1, space="PSUM") as ps_tr,
            tc.tile_pool(name="ps_mm", bufs=2, space="PSUM") as ps_mm,
        ):
            # ---------------- constants / singles ----------------
            # b_o enters the outproj PSUM accumulation as a rank-1 matmul
            # (ones-column x bo-row), so stores DMA straight from PSUM
            bo_sb = singles.tile([1, D], BF16, tag="bo", name="bo_sb")
            nc.gpsimd.dma_start(out=bo_sb, in_=ins["bo"])
            ones_col = singles.tile([1, P], BF16, tag="ones", name="ones_col")
            nc.gpsimd.memset(ones_col, 1.0)

            ident = singles.tile([P, P], BF16, tag="id", name="ident")
            make_identity(nc, ident)

            qt = singles.tile([P, PO, SC], BF16, tag="qt", name="qt")
            kt = singles.tile([P, PO, SC], BF16, tag="kt", name="kt")
            kt2 = singles.tile([P, PO, SC], BF16, tag="kt2", name="kt2")
            hcatT = singles.tile([P, PO, SC], BF16, tag="hct", name="hcatT")
            # V per head: [t', j', 64 V cols + ones col] -> denominator rides
            # the A@V matmul on free idx 64
            v_sb = [singles.tile([P, 16, 65], BF16, tag=f"v{hl}",
                                 name=f"v_sb{hl}") for hl in range(HPC)]
            for hl in range(HPC):
                nc.gpsimd.memset(v_sb[hl][:, :, 64:65], 1.0)

            # warmup fodder (keeps the PE p-state ramp warm during DMA waits)
            junk = singles.tile([P, 512], BF16, tag="junk", name="junk")
            nc.gpsimd.memset(junk, 0.0)

            # ---------------- input DMA emissions ----------------
            # Emission order == DMA service order (one pooled DMA device in
            # the cost model); order the critical first-scores chain first.
            xs = {}   # (name, s-half) -> tile  [P, KD, 256]
            ws = {}   # name -> tile [P, KD, D], loaded in f-halves
            xsrc = {"xq": ins["xqt"], "xk": ins["xkt"], "xv": ins["xvt"]}
            wsrc = {"wq": ins["wqt"], "wk": ins["wkt"], "wv": ins["wvt"],
                    "wo": ins["wot"]}
            for n, ap in wsrc.items():
                ws[n] = singles.tile([P, KD, D], BF16, tag=n, name=n)
            for n in xsrc:
                for sh in range(2):
                    xs[n, sh] = singles.tile([P, KD, 256], BF16,
                                             tag=f"{n}{sh}", name=f"{n}{sh}")

            def load_w(n, fh, quarter=False):
                src = wsrc[n].rearrange("(kd p) f -> p kd f", p=P)
                w = 256 if quarter else 512
                fs = slice(fh * w, (fh + 1) * w)
                nc.sync.dma_start(ws[n][:, :, fs], src[:, :, fs])

            def load_x(n, sh):
                src = xsrc[n].rearrange("(kd p) s -> p kd s", p=P)
                ss = slice(sh * 256, (sh + 1) * 256)
                nc.sync.dma_start(xs[n, sh], src[:, :, ss])

            def load_kt2(mlo, mhi, sh):
                """Grouped partition-rotated KT copy (kt2[p]=kt[(p+64)%128]).
                Placed in the SP DMA stream: its kt-copy waits block SP's
                in-order SEQ, which both throttles later (non-urgent) input
                loads and guarantees kt2 isn't starved on the DMA device."""
                ms = slice(mlo, mhi)
                ss = slice(sh * 256, (sh + 1) * 256)
                nc.sync.dma_start(kt2[0:64, ms, ss], kt[64:128, ms, ss])
                nc.sync.dma_start(kt2[64:128, ms, ss], kt[0:64, ms, ss])

            # critical chain first: first scores need wq-h0 + xq-h0 (Q mf0-3)
            # + wk quarters streaming. All remaining loads are emitted inside
            # quarter 0 (after the kt2-sh0 rotation DMAs) so the kt2 copies
            # get SP-queue priority over the long-slack input loads. kt2
            # emissions must FOLLOW their kt-copy producers (Tile deps track
            # emission order).
            load_w("wq", 0, True); load_x("xq", 0)
            load_x("xk", 0)
            load_w("wk", 0, True)
            load_w("wq", 1, True)
            load_w("wk", 1, True); load_w("wk", 2, True)
            load_w("wk", 3, True)

            def load_rest():
                load_w("wq", 1)
                load_w("wv", 0); load_x("xv", 0); load_w("wv", 1)
                load_x("xq", 1); load_x("xk", 1); load_x("xv", 1)
                load_w("wo", 0); load_w("wo", 1)

            # ---------------- PE warmup ----------------
            for _ in range(N_WARMUP):
                d_ps = ps_at.tile([P, 512], F32, tag="at", name="d_ps")
                nc.tensor.matmul(d_ps, junk[:, 0:P], junk, start=True,
                                 stop=True)

            # ---------------- projection units ----------------
            # proj_qk(t, mf, sh): QT/KT[f-chunk mf, s-half sh] = 8 kd matmuls
            # (free 256) + DVE copy. `part` splits the kd accumulation into
            # two 4-matmul drains so a unit never blocks score emission for
            # more than ~0.4us of PE time (ACT buffer is only ~1.8us deep).
            proj_ps = {}

            def proj_qk(which, mf, sh, part=None):
                w_t = ws["wq" if which == "q" else "wk"]
                x_t = xs[("xq" if which == "q" else "xk"), sh]
                dst = qt if which == "q" else kt
                ss = slice(sh * 256, (sh + 1) * 256)
                kds = range(KD) if part is None else \
                    range(part * 4, part * 4 + 4)
                if part in (None, 0):
                    proj_ps[which, mf, sh] = ps_mm.tile([P, 512], F32,
                                                        tag="mm", name="ps")
                ps = proj_ps[which, mf, sh]
                for kd in kds:
                    nc.tensor.matmul(
                        ps[:, 0:256], w_t[:, kd, mf * P:(mf + 1) * P],
                        x_t[:, kd, :], start=(kd == 0), stop=(kd == KD - 1))
                if part in (None, 1):
                    del proj_ps[which, mf, sh]
                    nc.vector.tensor_copy(dst[:, mf, ss], ps[:, 0:256])

            # V proj: V[s, f] for head hl, f-half nf (j' chunks 8nf..8nf+7)
            def v_proj(hl, nf, part=None):
                sh, so = hl // 2, (hl % 2) * 128
                kds = range(KD) if part is None else \
                    range(part * 4, part * 4 + 4)
                if part in (None, 0):
                    proj_ps["v", hl, nf] = ps_mm.tile([P, 512], F32,
                                                      tag="mm", name="ps")
                ps = proj_ps["v", hl, nf]
                for kd in kds:
                    nc.tensor.matmul(
                        ps, xs["xv", sh][:, kd, so:so + P],
                        ws["wv"][:, kd, nf * 512:(nf + 1) * 512],
                        start=(kd == 0), stop=(kd == KD - 1))
                if part in (None, 1):
                    del proj_ps["v", hl, nf]
                    nc.vector.tensor_copy(
                        v_sb[hl][:, nf * 8:(nf + 1) * 8, 0:64],
                        ps.rearrange("p (j k) -> p j k", k=64))

            # ---------------- attention pieces ----------------
            # per (head, quarter qi): quarter = (pp, a); j'-order: same-parity
            # first (kt source) then opposite (kt2, which lands ~2us later).
            def jorder(a):
                return [j for j in range(16) if j % 2 == a] + \
                       [j for j in range(16) if j % 2 != a]

            ex_tiles = {}  # (hl, qi, j') -> SBUF bf16 [128, 512] (or半 pair)

            def scores_exp(hl, qi, jp, half=None):
                """half=0/1: free-256 score tile covering po-chunk pair
                (4pp+2*half) — lets the first exps start before Q mf2-3
                project. ex_tiles holds a (loA, hiB) tuple in that case."""
                pp, a = qi // 2, qi % 2
                hs = slice(hl * P, (hl + 1) * P)
                pb = slice(64 * a, 64 * a + 64)
                src = kt if jp % 2 == a else kt2
                at = ps_at.tile([P, 512], F32, tag="at", name="at")
                if half is None:
                    po_s = slice(4 * pp, 4 * pp + 4)
                    at_v, wf = at, 512
                else:
                    po_s = slice(4 * pp + 2 * half, 4 * pp + 2 * half + 2)
                    at_v, wf = at[:, 0:256], 256
                nc.tensor.matmul(at_v, src[pb, jp // 2, hs],
                                 qt[pb, po_s, hs], start=True, stop=True)
                ex = exq.tile([P, 512], BF16, tag="ex", name="ex")
                nc.scalar.activation(ex[:, 0:wf], at_v, EXPF, scale=0.125)
                if half is None:
                    ex_tiles[hl, qi, jp] = ex
                else:
                    pair = ex_tiles.setdefault((hl, qi, jp), [None, None])
                    pair[half] = ex

            def ex_chunk(exv, c):
                if isinstance(exv, list):
                    return exv[c // 2][:, (c % 2) * P:(c % 2 + 1) * P]
                return exv[:, c * P:(c + 1) * P]

            o_tiles = {}  # (hl, qi) -> PSUM [128, 4, 128]

            def av(hl, qi, jp, first, last):
                # one accumulation group per o4 bank: start zeroes the whole
                # 2KB zero-region lazily, so only the very first matmul may
                # set start and only the very last sets stop
                exv = ex_tiles.pop((hl, qi, jp))
                if first:
                    o_tiles[hl, qi] = ps_o.tile([P, 4, P], F32, tag="o",
                                                name="o4")
                o4 = o_tiles[hl, qi]
                for c in range(4):
                    nc.tensor.matmul(o4[:, c, 0:65], ex_chunk(exv, c),
                                     v_sb[hl][:, jp, :],
                                     start=first and c == 0,
                                     stop=last and c == 3)

            hcp_tiles = {}  # (hl, po) -> SBUF bf16 [128, 2, 64]

            def norm(hl, qi):
                """recip + 4 per-partition scaling multiplies for quarter qi."""
                pp, a = qi // 2, qi % 2
                o4 = o_tiles.pop((hl, qi))
                nc.vector.reciprocal(o4[:, :, 64:65], o4[:, :, 64:65])
                for c in range(4):
                    po = 4 * pp + c
                    if (hl, po) not in hcp_tiles:
                        hcp_tiles[hl, po] = hcpp.tile([P, 2, 64], BF16,
                                                      tag="hcp", name="hcp")
                    nc.vector.tensor_scalar(
                        hcp_tiles[hl, po][:, a, :], o4[:, c, 0:64],
                        o4[:, c, 64:65], None, MULT)

            def transposes(hl, pp):
                hs = slice(hl * P, (hl + 1) * P)
                tps = ps_tr.tile([P, 4, P], BF16, tag="tr", name="tps")
                for c in range(4):
                    po = 4 * pp + c
                    hcp = hcp_tiles.pop((hl, po))
                    nc.tensor.transpose(tps[:, c, :],
                                        hcp.rearrange("p a d -> p (a d)"),
                                        ident)
                    nc.vector.tensor_copy(hcatT[:, po, hs], tps[:, c, :])

            op_ps = {}

            def outproj(hl, nf, poh):
                """Half of the po-contraction for (head, f-half nf)."""
                hs = slice(hl * P, (hl + 1) * P)
                fs = slice(nf * 512, (nf + 1) * 512)
                if poh == 0:
                    ps = ps_mm.tile([P, 512], F32, tag="mm", name="ps")
                    op_ps[hl, nf] = ps
                    # bias via rank-1 matmul opens the accumulation group
                    nc.tensor.matmul(ps, ones_col, bo_sb[:, fs],
                                     start=True, stop=False)
                ps = op_ps[hl, nf]
                for po in range(4 * poh, 4 * poh + 4):
                    nc.tensor.matmul(ps, hcatT[:, po, hs], ws["wo"][:, po, fs],
                                     start=False, stop=(po == PO - 1))
                if poh == 1:
                    del op_ps[hl, nf]
                    os_t = outp.tile([P, 512], F32, tag="os", name="os_t")
                    if hl == HPC - 1:
                        # tail: ACT is idle after the last exp
                        nc.scalar.copy(os_t, ps)
                    else:
                        nc.vector.tensor_copy(os_t, ps)
                    nc.sync.dma_start(out_ap[hs, fs], os_t)

            # ---------------- schedule ----------------
            # Pre-attention: Q mf0-3 + K mf0-3 (s-half 0); quarter 0 runs a
            # custom j'-order with K mf4-7 interleaved as their DMA halves
            # land. All other proj / v_proj / outproj work becomes filler
            # units drained inside the attention loop at fixed slots — PE
            # runs ~2x faster than ACT during attention, so fillers absorb
            # PE idle gaps without starving ACT (3 at-banks = ~1.8us of
            # buffered ACT work rides out short PE stalls).
            # AVs lag scores by TWO quarters so every ex tile and V half is
            # long since ready when its AV executes.
            proj_qk("q", 0, 0)
            proj_qk("q", 1, 0)
            proj_qk("k", 0, 0)
            proj_qk("q", 2, 0)
            proj_qk("q", 3, 0)

# Deferred work units, drained at fixed slots inside the
            # attention loop (PE outruns ACT there, so units absorb PE idle
            # gaps). Every unit is keyed and ensure()d right before its
            # first consumer is emitted — slot arithmetic is a performance
            # heuristic, never a correctness requirement.
            from collections import OrderedDict
            units = OrderedDict()

            def kp_sh1_tail(mf):
                proj_qk("k", mf, 1, 1)
                if mf == 3:
                    load_kt2(0, 4, 1)
                elif mf == 7:
                    load_kt2(4, 8, 1)

            def add_unit(kind, a, b):
                if kind == "qp":
                    units["qp", a, b, 0] = lambda: proj_qk("q", a, b, 0)
                    units["qp", a, b, 1] = lambda: proj_qk("q", a, b, 1)
                elif kind == "kp":
                    units["kp", a, b, 0] = lambda: proj_qk("k", a, b, 0)
                    units["kp", a, b, 1] = lambda: kp_sh1_tail(a)
                else:
                    units["vp", a, b, 0] = lambda: v_proj(a, b, 0)
                    units["vp", a, b, 1] = lambda: v_proj(a, b, 1)

            for mf in range(4, PO):
                add_unit("qp", mf, 0)
            add_unit("vp", 0, 0)
            add_unit("vp", 0, 1)
            add_unit("vp", 1, 0)
            add_unit("vp", 1, 1)
            for mf in range(PO):
                add_unit("kp", mf, 1)
                add_unit("qp", mf, 1)
                if mf == 2:
                    add_unit("vp", 2, 0)
                    add_unit("vp", 2, 1)
                if mf == 4:
                    add_unit("vp", 3, 0)
                    add_unit("vp", 3, 1)

            def ensure(*key):
                fn = units.pop(key, None)
                if fn is not None:
                    fn()

            def ensure2(kind, a, b):
                ensure(kind, a, b, 0)
                ensure(kind, a, b, 1)

            def fill(n=1):
                for _ in range(n):
                    if units:
                        units.pop(next(iter(units)))()

            def ensure_scores_deps(shl, sqi):
                """Emit any still-pending proj units whose outputs quarter
                (shl, sqi)'s scores read."""
                sh = shl // 2
                if sh == 1:
                    for mf in range(PO):
                        ensure2("kp", mf, 1)
                mlo = 4 * (sqi // 2)
                for mf in range(mlo, mlo + 4):
                    ensure2("qp", mf, sh)

            quarters = [(hl, qi) for hl in range(HPC) for qi in range(4)]
            jo_used = {}

            def post_av_block(ahl, aqi):
                norm(ahl, aqi)
                if aqi % 2 == 1:
                    transposes(ahl, aqi // 2)
                    if aqi == 3:
                        for nf, poh in ((0, 0), (0, 1), (1, 0), (1, 1)):
                            units["op", ahl, nf, poh] = (
                                lambda hl=ahl, nf=nf, poh=poh:
                                outproj(hl, nf, poh))

            for i, (hl, qi) in enumerate(quarters):
                avt = quarters[i - 2] if i >= 2 else None
                jo_av = jo_used[avt] if avt else None
                if i == 0:
                    # custom: quarter 0's kt-parity (even) j' interleaved
                    # with (a) K mf1-7 projections as the wk quarters land
                    # and (b) quarter 1's kt-parity (odd) j' — the K-proj
                    # phase is PE-bound at ~1.06us/score, so doubling the
                    # score stream here keeps ACT saturated. kt2 rotation
                    # DMAs then have a half-quarter of slack before the
                    # first kt2-parity j' needs them.
                    evens = [0, 2, 4, 6, 8, 10, 12, 14]
                    odds = [1, 3, 5, 7, 9, 11, 13, 15]
                    for idx in range(8):
                        scores_exp(hl, 0, evens[idx])
                        if idx < 7:
                            proj_qk("k", idx + 1, 0)
                        if idx == 2:
                            load_kt2(0, 4, 0)
                        elif idx == 6:
                            load_kt2(4, 8, 0)
                        elif idx == 7:
                            load_rest()
                        scores_exp(hl, 1, odds[idx])
                    for jp in odds:
                        scores_exp(hl, 0, jp)
                    jo_used[hl, 0] = evens + odds
                elif i == 1:
                    for idx, jp in enumerate(evens):
                        scores_exp(hl, 1, jp)
                        if idx % 2 == 1:
                            fill()
                    jo_used[hl, 1] = odds + evens
                else:
                    ensure_scores_deps(hl, qi)
                    if avt is not None and avt[1] == 0:
                        ensure2("vp", avt[0], 0)
                        ensure2("vp", avt[0], 1)
                    jo = jorder(qi % 2)
                    jo_used[hl, qi] = jo
                    for idx, jp in enumerate(jo):
                        scores_exp(hl, qi, jp)
                        if avt is not None:
                            av(*avt, jo_av[idx], idx == 0, idx == 15)
                        if idx % 3 == 2:
                            fill()
                if avt is not None:
                    post_av_block(*avt)

            # ---- tail ----
            # quarter (3,2): AVs + norm; head-3's po0-3 outproj partials can
            # already run (transposes(3,0) done after quarter (3,1)).
            thl, tqi = quarters[-2]
            jo_av = jo_used[thl, tqi]
            for idx, jp in enumerate(jo_av):
                av(thl, tqi, jp, idx == 0, idx == 15)
                fill()
            norm(thl, tqi)
            fill(len(units))
            outproj(thl, 0, 0)
            outproj(thl, 1, 0)

            # quarter (3,3): AVs, then a per-chunk pipelined finale so the
            # post-last-exp critical chain is as short as possible:
            # recip_c -> norm_c -> transpose_c -> copy_c -> outproj(po4+c),
            # then bias + store per f-half.
            thl, tqi = quarters[-1]
            jo_av = jo_used[thl, tqi]
            for idx, jp in enumerate(jo_av):
                av(thl, tqi, jp, idx == 0, idx == 15)
            o4 = o_tiles.pop((thl, tqi))
            hs = slice(thl * P, (thl + 1) * P)
            tps = ps_tr.tile([P, 4, P], BF16, tag="tr", name="tps")
            for c in range(4):
                po = 4 + c
                nc.vector.reciprocal(o4[:, c, 64:65], o4[:, c, 64:65])
                hcp = hcp_tiles.pop((thl, po))
                nc.vector.tensor_scalar(hcp[:, 1, :], o4[:, c, 0:64],
                                        o4[:, c, 64:65], None, MULT)
                nc.tensor.transpose(tps[:, c, :],
                                    hcp.rearrange("p a d -> p (a d)"), ident)
                nc.vector.tensor_copy(hcatT[:, po, hs], tps[:, c, :])
                for nf in range(2):
                    fs = slice(nf * 512, (nf + 1) * 512)
                    nc.tensor.matmul(op_ps[thl, nf], hcatT[:, po, hs],
                                     ws["wo"][:, po, fs],
                                     start=False, stop=(po == PO - 1))
            for nf in range(2):
                # one copy per engine so they run in parallel at the end
                fs = slice(nf * 512, (nf + 1) * 512)
                ps = op_ps.pop((thl, nf))
                os_t = outp.tile([P, 512], F32, tag="os", name="os_t")
                if nf == 0:
                    nc.vector.tensor_copy(os_t, ps)
                else:
                    nc.scalar.copy(os_t, ps)
                nc.sync.dma_start(out_ap[hs, fs], os_t)
    return nc


def build_program():
    nc = bacc.Bacc("TRN2", target_bir_lowering=False, debug=False,
                   enable_asserts=False, num_devices=NCORES)
    ins = {
        "xqt": nc.dram_tensor("xqt", [D, SC], BF16, kind="ExternalInput").ap(),
        "xkt": nc.dram_tensor("xkt", [D, SC], BF16, kind="ExternalInput").ap(),
        "xvt": nc.dram_tensor("xvt", [D, SC], BF16, kind="ExternalInput").ap(),
        "wqt": nc.dram_tensor("wqt", [D, D], BF16, kind="ExternalInput").ap(),
        "wkt": nc.dram_tensor("wkt", [D, D], BF16, kind="ExternalInput").ap(),
        "wvt": nc.dram_tensor("wvt", [D, D], BF16, kind="ExternalInput").ap(),
        "wot": nc.dram_tensor("wot", [D, D], BF16, kind="ExternalInput").ap(),
        "bo": nc.dram_tensor("bo", [1, D], BF16, kind="ExternalInput").ap(),
    }
    out_ap = nc.dram_tensor("out", [SC, D], F32, kind="ExternalOutput").ap()
    build_body(nc, out_ap, ins)
    nc.finalize()
    return nc


def make_in_maps(inputs):
    import ml_dtypes
    ndt = ml_dtypes.bfloat16
    Xq = np.asarray(inputs["X_q"], dtype=np.float32)
    Xk = np.asarray(inputs["X_k"], dtype=np.float32)
    Xv = np.asarray(inputs["X_v"], dtype=np.float32)
    wqt = np.ascontiguousarray(np.asarray(inputs["W_q"], np.float32).T).astype(ndt)
    wkt = np.ascontiguousarray(np.asarray(inputs["W_k"], np.float32).T).astype(ndt)
    wvt = np.ascontiguousarray(np.asarray(inputs["W_v"], np.float32).T).astype(ndt)
    wot = np.ascontiguousarray(np.asarray(inputs["W_o"], np.float32).T).astype(ndt)
    bo = np.asarray(inputs["b_o"], np.float32).reshape(1, D).astype(ndt)
    xt = {n: [np.ascontiguousarray(x[b].T).astype(ndt) for b in range(B)]
          for n, x in (("xqt", Xq), ("xkt", Xk), ("xvt", Xv))}
    in_maps = []
    for c in range(NCORES):
        b, g = divmod(c, NCORES // B)
        sl = slice(g * SC, (g + 1) * SC)
        in_maps.append({
            "xqt": np.ascontiguousarray(xt["xqt"][b][:, sl]),
            "xkt": np.ascontiguousarray(xt["xkt"][b][:, sl]),
            "xvt": np.ascontiguousarray(xt["xvt"][b][:, sl]),
            "wqt": wqt, "wkt": wkt, "wvt": wvt, "wot": wot, "bo": bo,
        })
    return in_maps


_NC_CACHE = {}


def _run(inputs, trace=False, trace_cores=None):
    from concourse.bass_utils import run_bass_kernel_spmd
    if MODE not in _NC_CACHE:
        _NC_CACHE[MODE] = build_program()
    nc = _NC_CACHE[MODE]
    in_maps = make_in_maps(inputs)
    res = run_bass_kernel_spmd(nc, in_maps, core_ids=list(range(NCORES)),
                               trace=trace, trace_cores=trace_cores)
    out = np.empty((B, S, D), dtype=np.float32)
    for c in range(NCORES):
        b, g = divmod(c, NCORES // B)
        out[b, g * SC:(g + 1) * SC, :] = res.results[c]["out"]
    return out, res


def kernel(**inputs):
    out, _ = _run(inputs, trace=False)
    return out


# revision 58
# speedup vs baseline: 1.2427x; 1.0071x over previous
"""MultiHeadAttention (faithful raw-reshape variant) on 8 trn2 NeuronCores.

Math (per batch b):
  Y  = Xq @ Wq.T            [S, D]
  Z  = Xk @ Wk.T            [S, D]
  V  = Xv @ Wv.T            [S, D]
  reshape (B,S,D)->(B,H,S,dk) is a *raw view*: head h <- rows [128h, 128h+128)
  of Y/Z/V; within the block, q = 16t + j maps to (row t, features 64j..64j+64).
  A  = softmax(Qh @ Kh.T / 8), O = A @ Vh, placed back into the same raw view,
  out = Hcat @ Wo.T + b_o.

Heads partition the *rows* of Y/Z/V, so work is fully independent across
(b, h): 32 tasks, 4 per core, no collectives.

Per-core device program (heads hl=0..3 over the core's 512 rows):
  QT/KT: transposed projections QT[f, s] ([128, 8, 512]); kt2 = partition-
         rotated KT copy so either j'-parity is on either partition half.
  scores: at[t', 128c+t] = sum_d K[(t',j'), d] Q[(t,j(c)), d]; one matmul per
         (head, quarter, j'): stationary kt-chunk [64, 128], moving qt [64,512].
  exp:   ACT, scale=1/8 fused, fp32 PSUM -> bf16 SBUF. ACT is the bottleneck
         engine (256 x ~0.6us); the schedule keeps it saturated.
  AV:    O[q, f] orientation: stationary ex-chunk [128, 128], moving
         V-chunk [128, 65] (64 V cols + ones col -> softmax denominator on
         free idx 64). 16 j'-chunks accumulate per PSUM tile [128, 4, 128].
         AVs lag scores by one quarter so PE never waits on ACT results.
  norm:  DVE reciprocal of denom cols + per-partition tensor_scalar multiply
         into bf16 hcp tiles [128 t, 2 side, 64 d] (= Hcat f-chunks).
  transpose: PE transpose (via identity) hcp -> hcatT chunks [128 f, 128 t].
  out:   out[t, g] = hcatT.T @ WoT, bias folded in as a rank-1 matmul;
         PSUM -> SBUF staging copy, then SP-queue DMA store.

The schedule targets the TimelineSim cost model: ACT (exp) is the bottleneck
at ~158us busy; scores/AVs/projections/outproj are interleaved so ACT has
zero idle between the first exp (~12.8us) and the last (~171us).
"""

import os

import numpy as np

import concourse.bass as bass
import concourse.mybir as mybir
import concourse.tile as tile
from concourse import bacc
from concourse.masks import make_identity

B, S, D = 2, 2048, 1024
H, DK = 16, 64
NCORES = 8
HPC = H // (NCORES // B)  # heads per core = 4
SC = HPC * 128            # s-rows per core = 512
P = 128
KD = D // P               # 8 contraction chunks
PO = D // P               # 8 feature chunks
F32 = mybir.dt.float32
BF16 = mybir.dt.bfloat16

MODE = "bf16"
N_WARMUP = int(os.environ.get("TRN_WARMUP", "14"))


def build_body(nc, out_ap, ins):
    """Emit the per-core program. ins: dict of DRAM APs."""
    EXPF = mybir.ActivationFunctionType.Exp
    MULT = mybir.AluOpType.mult
    ADD = mybir.AluOpType.add

    with tile.TileContext(nc) as tc:
        with (
            tc.tile_pool(name="singles", bufs=1) as singles,
            tc.tile_pool(name="exq", bufs=36) as exq,
            tc.tile_pool(name="hcp", bufs=8) as hcpp,
            tc.tile_pool(name="outp", bufs=2) as outp,
            tc.tile_pool(name="ps_at", bufs=3, space="PSUM") as ps_at,
            tc.tile_pool(name="ps_o", bufs=2, space="PSUM") as ps_o,
            tc.tile_pool(name="ps_tr", bufs=1, space="PSUM") as ps_tr,
            tc.tile_pool(name="ps_mm", bufs=2, space="PSUM") as ps_mm,
        ):
            # ---------------- constants / singles ----------------
            # b_o enters the outproj PSUM accumulation as a rank-1 matmul
            # (ones-column x bo-row), so stores DMA straight from PSUM
            bo_sb = singles.tile([1, D], BF16, tag="bo", name="bo_sb")
            nc.gpsimd.dma_start(out=bo_sb, in_=ins["bo"])
            ones_col = singles.tile([1, P], BF16, tag="ones", name="ones_col")
            nc.gpsimd.memset(ones_col, 1.0)

            ident = singles.tile([P, P], BF16, tag="id", name="ident")
            make_identity(nc, ident)

            qt = singles.tile([P, PO, SC], BF16, tag="qt", name="qt")
            kt = singles.tile([P, PO, SC], BF16, tag="kt", name="kt")
            kt2 = singles.tile([P, PO, SC], BF16, tag="kt2", name="kt2")
            hcatT = singles.tile([P, PO, SC], BF16, tag="hct", name="hcatT")
            # V per head: [t', j', 64 V cols + ones col] -> denominator rides
            # the A@V matmul on free idx 64
            v_sb = [singles.tile([P, 16, 65], BF16, tag=f"v{hl}",
                                 name=f"v_sb{hl}") for hl in range(HPC)]
            for hl in range(HPC):
                nc.gpsimd.memset(v_sb[hl][:, :, 64:65], 1.0)

            # warmup fodder (keeps the PE p-state ramp warm during DMA waits)
            junk = singles.tile([P, 512], BF16, tag="junk", name="junk")
            nc.gpsimd.memset(junk, 0.0)

            # ---------------- input DMA emissions ----------------
            # Emission order == DMA service order (one pooled DMA device in
            # the cost model); order the critical first-scores chain first.
            xs = {}   # (name, s-half) -> tile  [P, KD, 256]
            ws = {}   # name -> tile [P, KD, D], loaded in f-halves
            xsrc = {"xq": ins["xqt"], "xk": ins["xkt"], "xv": ins["xvt"]}
            wsrc = {"wq": ins["wqt"], "wk": ins["wkt"], "wv": ins["wvt"],
                    "wo": ins["wot"]}
            for n, ap in wsrc.items():
                ws[n] = singles.tile([P, KD, D], BF16, tag=n, name=n)
            for n in xsrc:
                for sh in range(2):
                    xs[n, sh] = singles.tile([P, KD, 256], BF16,
                                             tag=f"{n}{sh}", name=f"{n}{sh}")

            def load_w(n, fh, quarter=False):
                src = wsrc[n].rearrange("(kd p) f -> p kd f", p=P)
                w = 256 if quarter else 512
                fs = slice(fh * w, (fh + 1) * w)
                nc.sync.dma_start(ws[n][:, :, fs], src[:, :, fs])

            def load_x(n, sh):
                src = xsrc[n].rearrange("(kd p) s -> p kd s", p=P)
                ss = slice(sh * 256, (sh + 1) * 256)
                nc.sync.dma_start(xs[n, sh], src[:, :, ss])

            def load_kt2(mlo, mhi, sh):
                """Grouped partition-rotated KT copy (kt2[p]=kt[(p+64)%128]).
                Placed in the SP DMA stream: its kt-copy waits block SP's
                in-order SEQ, which both throttles later (non-urgent) input
                loads and guarantees kt2 isn't starved on the DMA device."""
                ms = slice(mlo, mhi)
                ss = slice(sh * 256, (sh + 1) * 256)
                nc.sync.dma_start(kt2[0:64, ms, ss], kt[64:128, ms, ss])
                nc.sync.dma_start(kt2[64:128, ms, ss], kt[0:64, ms, ss])

            # critical chain first: first scores need wq-h0 + xq-h0 (Q mf0-3)
            # + wk quarters streaming. All remaining loads are emitted inside
            # quarter 0 (after the kt2-sh0 rotation DMAs) so the kt2 copies
            # get SP-queue priority over the long-slack input loads. kt2
            # emissions must FOLLOW their kt-copy producers (Tile deps track
            # emission order).
            load_w("wq", 0, True); load_x("xq", 0)
            load_x("xk", 0)
            load_w("wk", 0, True)
            load_w("wq", 1, True)
            load_w("wk", 1, True); load_w("wk", 2, True)
            load_w("wk", 3, True)

            def load_rest():
                load_w("wq", 1)
                load_w("wv", 0); load_x("xv", 0); load_w("wv", 1)
                load_x("xq", 1); load_x("xk", 1); load_x("xv", 1)
                load_w("wo", 0); load_w("wo", 1)

            # ---------------- PE warmup ----------------
            for _ in range(N_WARMUP):
                d_ps = ps_at.tile([P, 512], F32, tag="at", name="d_ps")
                nc.tensor.matmul(d_ps, junk[:, 0:P], junk, start=True,
                                 stop=True)

            # ---------------- projection units ----------------
            # proj_qk(t, mf, sh): QT/KT[f-chunk mf, s-half sh] = 8 kd matmuls
            # (free 256) + DVE copy. `part` splits the kd accumulation into
            # two 4-matmul drains so a unit never blocks score emission for
            # more than ~0.4us of PE time (ACT buffer is only ~1.8us deep).
            proj_ps = {}

            def proj_qk(which, mf, sh, part=None):
                w_t = ws["wq" if which == "q" else "wk"]
                x_t = xs[("xq" if which == "q" else "xk"), sh]
                dst = qt if which == "q" else kt
                ss = slice(sh * 256, (sh + 1) * 256)
                kds = range(KD) if part is None else \
                    range(part * 4, part * 4 + 4)
                if part in (None, 0):
                    proj_ps[which, mf, sh] = ps_mm.tile([P, 512], F32,
                                                        tag="mm", name="ps")
                ps = proj_ps[which, mf, sh]
                for kd in kds:
                    nc.tensor.matmul(
                        ps[:, 0:256], w_t[:, kd, mf * P:(mf + 1) * P],
                        x_t[:, kd, :], start=(kd == 0), stop=(kd == KD - 1))
                if part in (None, 1):
                    del proj_ps[which, mf, sh]
                    nc.vector.tensor_copy(dst[:, mf, ss], ps[:, 0:256])

            # V proj: V[s, f] for head hl, f-half nf (j' chunks 8nf..8nf+7)
            def v_proj(hl, nf, part=None):
                sh, so = hl // 2, (hl % 2) * 128
                kds = range(KD) if part is None else \
                    range(part * 4, part * 4 + 4)
                if part in (None, 0):
                    proj_ps["v", hl, nf] = ps_mm.tile([P, 512], F32,
                                                      tag="mm", name="ps")
                ps = proj_ps["v", hl, nf]
                for kd in kds:
                    nc.tensor.matmul(
                        ps, xs["xv", sh][:, kd, so:so + P],
                        ws["wv"][:, kd, nf * 512:(nf + 1) * 512],
                        start=(kd == 0), stop=(kd == KD - 1))
                if part in (None, 1):
                    del proj_ps["v", hl, nf]
                    nc.vector.tensor_copy(
                        v_sb[hl][:, nf * 8:(nf + 1) * 8, 0:64],
                        ps.rearrange("p (j k) -> p j k", k=64))

            # ---------------- attention pieces ----------------
            # per (head, quarter qi): quarter = (pp, a); j'-order: same-parity
            # first (kt source) then opposite (kt2, which lands ~2us later).
            def jorder(a):
                return [j for j in range(16) if j % 2 == a] + \
                       [j for j in range(16) if j % 2 != a]

            ex_tiles = {}  # (hl, qi, j') -> SBUF bf16 [128, 512] (or半 pair)

            def scores_exp(hl, qi, jp, half=None):
                """half=0/1: free-256 score tile covering po-chunk pair
                (4pp+2*half) — lets the first exps start before Q mf2-3
                project. ex_tiles holds a (loA, hiB) tuple in that case."""
                pp, a = qi // 2, qi % 2
                hs = slice(hl * P, (hl + 1) * P)
                pb = slice(64 * a, 64 * a + 64)
                src = kt if jp % 2 == a else kt2
                at = ps_at.tile([P, 512], F32, tag="at", name="at")
                if half is None:
                    po_s = slice(4 * pp, 4 * pp + 4)
                    at_v, wf = at, 512
                else:
                    po_s = slice(4 * pp + 2 * half, 4 * pp + 2 * half + 2)
                    at_v, wf = at[:, 0:256], 256
                nc.tensor.matmul(at_v, src[pb, jp // 2, hs],
                                 qt[pb, po_s, hs], start=True, stop=True)
                ex = exq.tile([P, 512], BF16, tag="ex", name="ex")
                nc.scalar.activation(ex[:, 0:wf], at_v, EXPF, scale=0.125)
                if half is None:
                    ex_tiles[hl, qi, jp] = ex
                else:
                    pair = ex_tiles.setdefault((hl, qi, jp), [None, None])
                    pair[half] = ex

            def ex_chunk(exv, c):
                if isinstance(exv, list):
                    return exv[c // 2][:, (c % 2) * P:(c % 2 + 1) * P]
                return exv[:, c * P:(c + 1) * P]

            o_tiles = {}  # (hl, qi) -> PSUM [128, 4, 128]

            def av(hl, qi, jp, first, last):
                # one accumulation group per o4 bank: start zeroes the whole
                # 2KB zero-region lazily, so only the very first matmul may
                # set start and only the very last sets stop
                exv = ex_tiles.pop((hl, qi, jp))
                if first:
                    o_tiles[hl, qi] = ps_o.tile([P, 4, P], F32, tag="o",
                                                name="o4")
                o4 = o_tiles[hl, qi]
                for c in range(4):
                    nc.tensor.matmul(o4[:, c, 0:65], ex_chunk(exv, c),
                                     v_sb[hl][:, jp, :],
                                     start=first and c == 0,
                                     stop=last and c == 3)

            hcp_tiles = {}  # (hl, po) -> SBUF bf16 [128, 2, 64]

            def norm(hl, qi):
                """recip + 4 per-partition scaling multiplies for quarter qi."""
                pp, a = qi // 2, qi % 2
                o4 = o_tiles.pop((hl, qi))
                nc.vector.reciprocal(o4[:, :, 64:65], o4[:, :, 64:65])
                for c in range(4):
                    po = 4 * pp + c
                    if (hl, po) not in hcp_tiles:
                        hcp_tiles[hl, po] = hcpp.tile([P, 2, 64], BF16,
                                                      tag="hcp", name="hcp")
                    nc.vector.tensor_scalar(
                        hcp_tiles[hl, po][:, a, :], o4[:, c, 0:64],
                        o4[:, c, 64:65], None, MULT)

            def transposes(hl, pp):
                hs = slice(hl * P, (hl + 1) * P)
                tps = ps_tr.tile([P, 4, P], BF16, tag="tr", name="tps")
                for c in range(4):
                    po = 4 * pp + c
                    hcp = hcp_tiles.pop((hl, po))
                    nc.tensor.transpose(tps[:, c, :],
                                        hcp.rearrange("p a d -> p (a d)"),
                                        ident)
                    nc.vector.tensor_copy(hcatT[:, po, hs], tps[:, c, :])

            op_ps = {}

            def outproj(hl, nf, poh):
                """Half of the po-contraction for (head, f-half nf)."""
                hs = slice(hl * P, (hl + 1) * P)
                fs = slice(nf * 512, (nf + 1) * 512)
                if poh == 0:
                    ps = ps_mm.tile([P, 512], F32, tag="mm", name="ps")
                    op_ps[hl, nf] = ps
                    # bias via rank-1 matmul opens the accumulation group
                    nc.tensor.matmul(ps, ones_col, bo_sb[:, fs],
                                     start=True, stop=False)
                ps = op_ps[hl, nf]
                for po in range(4 * poh, 4 * poh + 4):
                    nc.tensor.matmul(ps, hcatT[:, po, hs], ws["wo"][:, po, fs],
                                     start=False, stop=(po == PO - 1))
                if poh == 1:
                    del op_ps[hl, nf]
                    os_t = outp.tile([P, 512], F32, tag="os", name="os_t")
                    if hl == HPC - 1:
                        # tail: ACT is idle after the last exp
                        nc.scalar.copy(os_t, ps)
                    else:
                        nc.vector.tensor_copy(os_t, ps)
                    nc.sync.dma_start(out_ap[hs, fs], os_t)

            # ---------------- schedule ----------------
            # Pre-attention: Q mf0-3 + K mf0-3 (s-half 0); quarter 0 runs a
            # custom j'-order with K mf4-7 interleaved as their DMA halves
            # land. All other proj / v_proj / outproj work becomes filler
            # units drained inside the attention loop at fixed slots — PE
            # runs ~2x faster than ACT during attention, so fillers absorb
            # PE idle gaps without starving ACT (3 at-banks = ~1.8us of
            # buffered ACT work rides out short PE stalls).
            # AVs lag scores by TWO quarters so every ex tile and V half is
            # long since ready when its AV executes.
            proj_qk("q", 0, 0)
            proj_qk("q", 1, 0)
            proj_qk("k", 0, 0)
            proj_qk("q", 2, 0)
            proj_qk("q", 3, 0)

# Deferred work units, drained at fixed slots inside the
            # attention loop (PE outruns ACT there, so units absorb PE idle
            # gaps). Every unit is keyed and ensure()d right before its
            # first consumer is emitted — slot arithmetic is a performance
            # heuristic, never a correctness requirement.
            from collections import OrderedDict
            units = OrderedDict()

            def kp_sh1_tail(mf):
                proj_qk("k", mf, 1, 1)
                if mf == 3:
                    load_kt2(0, 4, 1)
                elif mf == 7:
                    load_kt2(4, 8, 1)

            def add_unit(kind, a, b):
                if kind == "qp":
                    units["qp", a, b, 0] = lambda: proj_qk("q", a, b, 0)
                    units["qp", a, b, 1] = lambda: proj_qk("q", a, b, 1)
                elif kind == "kp":
                    units["kp", a, b, 0] = lambda: proj_qk("k", a, b, 0)
                    units["kp", a, b, 1] = lambda: kp_sh1_tail(a)
                else:
                    units["vp", a, b, 0] = lambda: v_proj(a, b, 0)
                    units["vp", a, b, 1] = lambda: v_proj(a, b, 1)

            for mf in range(4, PO):
                add_unit("qp", mf, 0)
            add_unit("vp", 0, 0)
            add_unit("vp", 0, 1)
            add_unit("vp", 1, 0)
            add_unit("vp", 1, 1)
            for mf in range(PO):
                add_unit("kp", mf, 1)
                add_unit("qp", mf, 1)
                if mf == 2:
                    add_unit("vp", 2, 0)
                    add_unit("vp", 2, 1)
                if mf == 4:
                    add_unit("vp", 3, 0)
                    add_unit("vp", 3, 1)

            def ensure(*key):
                fn = units.pop(key, None)
                if fn is not None:
                    fn()

            def ensure2(kind, a, b):
                ensure(kind, a, b, 0)
                ensure(kind, a, b, 1)

            def fill(n=1):
                for _ in range(n):
                    if units:
                        units.pop(next(iter(units)))()

            def ensure_scores_deps(shl, sqi):
                """Emit any still-pending proj units whose outputs quarter
                (shl, sqi)'s scores read."""
                sh = shl // 2
                if sh == 1:
                    for mf in range(PO):
                        ensure2("kp", mf, 1)
                mlo = 4 * (sqi // 2)
                for mf in range(mlo, mlo + 4):
                    ensure2("qp", mf, sh)

            quarters = [(hl, qi) for hl in range(HPC) for qi in range(4)]
            jo_used = {}

            def post_av_block(ahl, aqi):
                norm(ahl, aqi)
                if aqi % 2 == 1:
                    transposes(ahl, aqi // 2)
                    if aqi == 3:
                        for nf, poh in ((0, 0), (0, 1), (1, 0), (1, 1)):
                            units["op", ahl, nf, poh] = (
                                lambda hl=ahl, nf=nf, poh=poh:
                                outproj(hl, nf, poh))

            for i, (hl, qi) in enumerate(quarters):
                avt = quarters[i - 2] if i >= 2 else None
                jo_av = jo_used[avt] if avt else None
                if i == 0:
                    # custom: quarter 0's kt-parity (even) j' interleaved
                    # with (a) K mf1-7 projections as the wk quarters land
                    # and (b) quarter 1's kt-parity (odd) j' — the K-proj
                    # phase is PE-bound at ~1.06us/score, so doubling the
                    # score stream here keeps ACT saturated. kt2 rotation
                    # DMAs then have a half-quarter of slack before the
                    # first kt2-parity j' needs them.
                    evens = [0, 2, 4, 6, 8, 10, 12, 14]
                    odds = [1, 3, 5, 7, 9, 11, 13, 15]
                    for idx in range(8):
                        scores_exp(hl, 0, evens[idx])
                        if idx < 7:
                            proj_qk("k", idx + 1, 0)
                        if idx == 2:
                            load_kt2(0, 4, 0)
                        elif idx == 6:
                            load_kt2(4, 8, 0)
                        elif idx == 7:
                            load_rest()
                        scores_exp(hl, 1, odds[idx])
                    for jp in odds:
                        scores_exp(hl, 0, jp)
                    jo_used[hl, 0] = evens + odds
                elif i == 1:
                    for idx, jp in enumerate(evens):
                        scores_exp(hl, 1, jp)
                        if idx % 2 == 1:
                            fill()
                    jo_used[hl, 1] = odds + evens
                else:
                    ensure_scores_deps(hl, qi)
                    if avt is not None and avt[1] == 0:
                        ensure2("vp", avt[0], 0)
                        ensure2("vp", avt[0], 1)
                    jo = jorder(qi % 2)
                    jo_used[hl, qi] = jo
                    for idx, jp in enumerate(jo):
                        scores_exp(hl, qi, jp)
                        if avt is not None:
                            av(*avt, jo_av[idx], idx == 0, idx == 15)
                        if idx % 3 == 2:
                            fill()
                if avt is not None:
                    post_av_block(*avt)

            # ---- tail ----
            # quarter (3,2): AVs + norm; head-3's po0-3 outproj partials can
            # already run (transposes(3,0) done after quarter (3,1)).
            thl, tqi = quarters[-2]
            jo_av = jo_used[thl, tqi]
            for idx, jp in enumerate(jo_av):
                av(thl, tqi, jp, idx == 0, idx == 15)
                fill()
            norm(thl, tqi)
            fill(len(units))
            outproj(thl, 0, 0)
            outproj(thl, 1, 0)

            # quarter (3,3): AVs, then a per-chunk pipelined finale so the
            # post-last-exp critical chain is as short as possible:
            # recip_c -> norm_c -> transpose_c -> copy_c -> outproj(po4+c),
            # then bias + store per f-half.
            thl, tqi = quarters[-1]
            jo_av = jo_used[thl, tqi]
            for idx, jp in enumerate(jo_av):
                av(thl, tqi, jp, idx == 0, idx == 15)
            o4 = o_tiles.pop((thl, tqi))
            hs = slice(thl * P, (thl + 1) * P)
            tps = ps_tr.tile([P, 4, P], BF16, tag="tr", name="tps")
# phase 1: per-chunk normalize -> transpose -> copy-out. The
            # outproj matmuls are deferred to phase 2: PE executes in
            # emission order, so a matmul between transposes would stall the
            # next transpose behind it.
            def fin_mm(po):
                for nf in range(2):
                    fs = slice(nf * 512, (nf + 1) * 512)
                    nc.tensor.matmul(op_ps[thl, nf], hcatT[:, po, hs],
                                     ws["wo"][:, po, fs],
                                     start=False, stop=(po == PO - 1))

            for c in range(4):
                po = 4 + c
                nc.vector.reciprocal(o4[:, c, 64:65], o4[:, c, 64:65])
                hcp = hcp_tiles.pop((thl, po))
                nc.vector.tensor_scalar(hcp[:, 1, :], o4[:, c, 0:64],
                                        o4[:, c, 64:65], None, MULT)
                nc.tensor.transpose(tps[:, c, :],
                                    hcp.rearrange("p a d -> p (a d)"), ident)
                nc.vector.tensor_copy(hcatT[:, po, hs], tps[:, c, :])
                if c >= 2:
                    fin_mm(4 + c - 2)
            fin_mm(6)
            fin_mm(7)
            for nf in range(2):
                # one copy per engine so they run in parallel at the end
                fs = slice(nf * 512, (nf + 1) * 512)
                ps = op_ps.pop((thl, nf))
                os_t = outp.tile([P, 512], F32, tag="os", name="os_t")
                if nf == 0:
                    nc.vector.tensor_copy(os_t, ps)
                else:
                    nc.scalar.copy(os_t, ps)
                nc.sync.dma_start(out_ap[hs, fs], os_t)
    return nc


def build_program():
    nc = bacc.Bacc("TRN2", target_bir_lowering=False, debug=False,
                   enable_asserts=False, num_devices=NCORES)
    ins = {
        "xqt": nc.dram_tensor("xqt", [D, SC], BF16, kind="ExternalInput").ap(),
        "xkt": nc.dram_tensor("xkt", [D, SC], BF16, kind="ExternalInput").ap(),
        "xvt": nc.dram_tensor("xvt", [D, SC], BF16, kind="ExternalInput").ap(),
        "wqt": nc.dram_tensor("wqt", [D, D], BF16, kind="ExternalInput").ap(),
        "wkt": nc.dram_tensor("wkt", [D, D], BF16, kind="ExternalInput").ap(),
        "wvt": nc.dram_tensor("wvt", [D, D], BF16, kind="ExternalInput").ap(),
        "wot": nc.dram_tensor("wot", [D, D], BF16, kind="ExternalInput").ap(),
        "bo": nc.dram_tensor("bo", [1, D], BF16, kind="ExternalInput").ap(),
    }
    out_ap = nc.dram_tensor("out", [SC, D], F32, kind="ExternalOutput").ap()
    build_body(nc, out_ap, ins)
    nc.finalize()
    return nc


def make_in_maps(inputs):
    import ml_dtypes
    ndt = ml_dtypes.bfloat16
    Xq = np.asarray(inputs["X_q"], dtype=np.float32)
    Xk = np.asarray(inputs["X_k"], dtype=np.float32)
    Xv = np.asarray(inputs["X_v"], dtype=np.float32)
    wqt = np.ascontiguousarray(np.asarray(inputs["W_q"], np.float32).T).astype(ndt)
    wkt = np.ascontiguousarray(np.asarray(inputs["W_k"], np.float32).T).astype(ndt)
    wvt = np.ascontiguousarray(np.asarray(inputs["W_v"], np.float32).T).astype(ndt)
    wot = np.ascontiguousarray(np.asarray(inputs["W_o"], np.float32).T).astype(ndt)
    bo = np.asarray(inputs["b_o"], np.float32).reshape(1, D).astype(ndt)
    xt = {n: [np.ascontiguousarray(x[b].T).astype(ndt) for b in range(B)]
          for n, x in (("xqt", Xq), ("xkt", Xk), ("xvt", Xv))}
    in_maps = []
    for c in range(NCORES):
        b, g = divmod(c, NCORES // B)
        sl = slice(g * SC, (g + 1) * SC)
        in_maps.append({
            "xqt": np.ascontiguousarray(xt["xqt"][b][:, sl]),
            "xkt": np.ascontiguousarray(xt["xkt"][b][:, sl]),
            "xvt": np.ascontiguousarray(xt["xvt"][b][:, sl]),
            "wqt": wqt, "wkt": wkt, "wvt": wvt, "wot": wot, "bo": bo,
        })
    return in_maps


_NC_CACHE = {}


def _run(inputs, trace=False, trace_cores=None):
    from concourse.bass_utils import run_bass_kernel_spmd
    if MODE not in _NC_CACHE:
        _NC_CACHE[MODE] = build_program()
    nc = _NC_CACHE[MODE]
    in_maps = make_in_maps(inputs)
    res = run_bass_kernel_spmd(nc, in_maps, core_ids=list(range(NCORES)),
                               trace=trace, trace_cores=trace_cores)
    out = np.empty((B, S, D), dtype=np.float32)
    for c in range(NCORES):
        b, g = divmod(c, NCORES // B)
        out[b, g * SC:(g + 1) * SC, :] = res.results[c]["out"]
    return out, res


def kernel(**inputs):
    out, _ = _run(inputs, trace=False)
    return out
